# revision 1
# baseline (speedup 1.0000x reference)
"""CLIP encoder layer (LN -> causal MHA -> residual -> LN -> quickGELU MLP -> residual)
on 8 Trainium2 NeuronCores, SPMD via bass/Tile.

Sharding: 8 shards = 4 batches x 2 query-groups. Core c handles batch c//2 with
parity s = c%2. Each core recomputes LN1 + K/V for the full sequence of its batch
(no collectives). Query rows are split zigzag so causal attention work is balanced
AND the program is shard-uniform: all parity differences live in DATA (a host-side
token permutation of x, the causal masks, and output row assembly).

Local token order (per core):
  s=0: [g512:1024, g0:512, g1536:2048, g1024:1536]   s=1: identity
Own query rows are always local [512, 1536). Attention q-block l=0 (local cols
512:1024) uses k-tiles 0..7 (all 8 masked via data); l=1 (local 1024:1536) uses
k-tiles 0..15 (tiles 8..15 masked via data).

Layouts: activations feature-major [d, tokens] for GEMMs; LayerNorm token-major
with PE transposes between. Score-path matmuls (h1/Wq/Wk/Wv/Q/K) run float32r
(FP22, 1 cycle/row at N=512); V/P, out-proj and the MLP run bf16. All weights and
masks are pre-permuted on the host into the exact SBUF layout so every DMA is one
contiguous run per partition; DMA issue is spread over the sync/gpsimd/scalar
queues. LN affine params, the 1/sqrt(hd) q-scale and quickGELU's 1.702 factors
are folded into weights/biases on the host.
"""

import sys

sys.path.insert(0, "/opt/trn_rl_repo")

import numpy as np
import ml_dtypes

import concourse.bass as bass
import concourse.mybir as mybir
import concourse.tile as tile
from concourse import bacc
from concourse.bass_utils import run_bass_kernel_spmd
from concourse.masks import make_identity

B, S, D = 4, 2048, 1024
H, HD = 16, 64
DFF = 4 * D
NCORES = 8
EPS = 1e-5
OWN = 1024          # own query rows per core, local rows [512, 1536)
QOFF = 512          # local col offset of own queries
F32 = mybir.dt.float32
BF16 = mybir.dt.bfloat16
F32R = mybir.dt.float32r
ALU = mybir.AluOpType
AF = mybir.ActivationFunctionType

_CACHE = {}


def _bcast(ap1d, p=128):
    """[N] DRAM vector viewed as [p, N] with partition-step 0 (broadcast load)."""
    return bass.AP(tensor=ap1d.tensor, offset=ap1d.offset, ap=[[0, p]] + list(ap1d.ap))


def _build_program():
    nc = bacc.Bacc("TRN2", target_bir_lowering=False, debug=False,
                   num_devices=NCORES)

    t = {}
    t["xb"] = nc.dram_tensor("xb", [S, D], F32, kind="ExternalInput").ap()
    t["wk8"] = nc.dram_tensor("wk8", [8, 128, D], F32R, kind="ExternalInput").ap()
    t["wq8"] = nc.dram_tensor("wq8", [8, 128, D], F32R, kind="ExternalInput").ap()
    t["wvd"] = nc.dram_tensor("wvd", [128, 2, 8, 512], F32R, kind="ExternalInput").ap()
    t["wod"] = nc.dram_tensor("wod", [128, 8 * D], BF16, kind="ExternalInput").ap()
    t["w1t"] = nc.dram_tensor("w1t", [32, 128, D], BF16, kind="ExternalInput").ap()
    t["w2d"] = nc.dram_tensor("w2d", [128, 32 * D], BF16, kind="ExternalInput").ap()
    t["mskd"] = nc.dram_tensor("mskd", [128, 16 * 512], BF16, kind="ExternalInput").ap()
    t["bqd"] = nc.dram_tensor("bqd", [128, 8], F32, kind="ExternalInput").ap()
    t["bkd"] = nc.dram_tensor("bkd", [128, 8], F32, kind="ExternalInput").ap()
    t["b1d"] = nc.dram_tensor("b1d", [128, 32], F32, kind="ExternalInput").ap()
    for b_ in ("bv", "bo", "b2"):
        t[b_] = nc.dram_tensor(b_, [D], F32, kind="ExternalInput").ap()
    t["yo"] = nc.dram_tensor("yo", [OWN, D], F32, kind="ExternalOutput").ap()
    # DRAM scratch: K spill split per m-tile to keep cross-phase deps narrow
    t["kfm"] = [nc.dram_tensor(f"kfm{m}", [128, S], F32R).ap() for m in range(8)]
    t["y_d"] = nc.dram_tensor("y_scr", [OWN, D], F32).ap()

    with tile.TileContext(nc) as tc:
        _body(nc, tc, t)
    nc.compile()
    return nc


def _ln_normalize(nc, stat, eps_t, x_t, out_t, tag):
    """LayerNorm (affine folded into downstream weights): out = (x-mu)*rstd.
    Stats on DVE, the big normalize pass on ACT (Identity, scale/bias per-row)."""
    st = stat.tile([128, 2, 6], F32, tag=f"{tag}st")
    for g in range(2):
        nc.vector.bn_stats(out=st[:, g, :], in_=x_t[:, g * 512:(g + 1) * 512])
    mv = stat.tile([128, 2], F32, tag=f"{tag}mv")
    nc.vector.bn_aggr(out=mv, in_=st)
    rstd = stat.tile([128, 1], F32, tag=f"{tag}rs")
    nc.scalar.activation(out=rstd, in_=mv[:, 1:2], func=AF.Sqrt, bias=eps_t, scale=1.0)
    nc.vector.reciprocal(out=rstd, in_=rstd)
    nmu = stat.tile([128, 1], F32, tag=f"{tag}nm")
    nc.vector.tensor_scalar(out=nmu, in0=mv[:, 0:1], scalar1=rstd, scalar2=-1.0,
                            op0=ALU.mult, op1=ALU.mult)
    nc.scalar.activation(out=out_t, in_=x_t, func=AF.Identity, bias=nmu, scale=rstd)


def _body(nc, tc, t):
    with tc.tile_pool(name="const", bufs=1) as const:
        ident = const.tile([128, 128], F32)
        make_identity(nc, ident)
        eps_t = const.tile([128, 1], F32)
        nc.vector.memset(eps_t, EPS)
        onesrc = const.tile([128, 256], F32)
        nc.vector.memset(onesrc, 1.0)
        ones64 = const.tile([1, 64], F32R)
        nc.vector.tensor_copy(out=ones64, in_=onesrc[0:1, 0:64])
        bq_t = const.tile([128, 8], F32)
        nc.sync.dma_start(out=bq_t, in_=t["bqd"])
        bk_t = const.tile([128, 8], F32)
        nc.sync.dma_start(out=bk_t, in_=t["bkd"])
        b1s_t = const.tile([128, 32], F32)
        nc.sync.dma_start(out=b1s_t, in_=t["b1d"])
        b2_bc = const.tile([128, D], F32)
        nc.sync.dma_start(out=b2_bc, in_=_bcast(t["b2"]))

        with tc.tile_pool(name="h2p", bufs=1) as h2p:
            h2sb = h2p.tile([128, 8, OWN], BF16)  # ln2 out, feature-major

            with tc.tile_pool(name="ctxp", bufs=1) as ctxp:
                ctx_fm = ctxp.tile([128, 8, OWN], BF16)

                with tc.tile_pool(name="vaugp", bufs=1) as vaugp:
                    vaug = vaugp.tile([128, 16, H, 65], BF16)
                    nc.vector.tensor_copy(
                        out=vaug[:, :, :, 64:65],
                        in_=onesrc.rearrange("p (a b c) -> p a b c", a=16, b=16))

                    with tc.tile_pool(name="qfm", bufs=1) as qpool:
                        q_fm = qpool.tile([128, 8, OWN], F32R)

                        # ===== P0: LN1 + transpose to h1_fm =====
                        with tc.tile_pool(name="p01", bufs=1) as p01:
                            h1_fm = p01.tile([128, 8, S], F32R)

                            with tc.tile_pool(name="p0work", bufs=3) as wrk, \
                                 tc.tile_pool(name="p0x", bufs=3) as xpool, \
                                 tc.tile_pool(name="p0stat", bufs=6) as stat, \
                                 tc.tile_pool(name="p0psum", bufs=4, space="PSUM") as psT:
                                for tt in range(S // 128):
                                    x_t = xpool.tile([128, D], F32, tag="x")
                                    nc.sync.dma_start(
                                        out=x_t, in_=t["xb"][tt * 128:(tt + 1) * 128, :])
                                    h1_t = wrk.tile([128, D], F32, tag="h1")
                                    _ln_normalize(nc, stat, eps_t, x_t, h1_t, "a")
                                    for dp in range(4):
                                        pt = psT.tile([128, 2, 128], F32, tag="pt")
                                        for j in range(2):
                                            nc.tensor.transpose(
                                                pt[:, j, :],
                                                h1_t[:, (2 * dp + j) * 128:(2 * dp + j + 1) * 128],
                                                ident)
                                        dst = h1_fm[:, 2 * dp:2 * dp + 2,
                                                    tt * 128:(tt + 1) * 128]
                                        if dp % 2 == 0:
                                            nc.vector.tensor_copy(out=dst, in_=pt)
                                        else:
                                            nc.scalar.copy(out=dst, in_=pt)

                            # ===== P1a: V projection -> vaug (bf16, SBUF) =====
                            with tc.tile_pool(name="wvpool", bufs=1) as wvpool, \
                                 tc.tile_pool(name="vpsum", bufs=3, space="PSUM") as vps:
                                bv_bc = wvpool.tile([128, D], F32, tag="bvbc")
                                nc.sync.dma_start(out=bv_bc, in_=_bcast(t["bv"]))
                                for fb in range(2):
                                    wvh = wvpool.tile([128, 8, 512], F32R, tag="wv")
                                    nc.sync.dma_start(out=wvh, in_=t["wvd"][:, fb, :, :])
                                    for tt in range(S // 128):
                                        ps = vps.tile([128, 512], F32, tag="ps")
                                        for kt in range(8):
                                            nc.tensor.matmul(
                                                ps, h1_fm[:, kt, tt * 128:(tt + 1) * 128],
                                                wvh[:, kt, :],
                                                start=(kt == 0), stop=(kt == 7))
                                        nc.vector.tensor_add(
                                            out=vaug[:, tt, fb * 8:(fb + 1) * 8, 0:64],
                                            in0=ps.rearrange("p (h f) -> p h f", h=8),
                                            in1=bv_bc[:, fb * 512:(fb + 1) * 512].rearrange(
                                                "p (h f) -> p h f", h=8))

                            # ===== P1b: K -> per-m DRAM spill; P1c: Q =====
                            with tc.tile_pool(name="wstr", bufs=3) as wstr, \
                                 tc.tile_pool(name="kout", bufs=3) as kout, \
                                 tc.tile_pool(name="kpsum", bufs=3, space="PSUM") as kps:
                                for m in range(8):
                                    wkm = wstr.tile([128, 8, 128], F32R, tag="w")
                                    nc.sync.dma_start(
                                        out=wkm,
                                        in_=t["wk8"][m].rearrange("p (t n) -> p t n", t=8))
                                    for qb in range(4):
                                        ps = kps.tile([128, 512], F32, tag="ps")
                                        for kt in range(8):
                                            nc.tensor.matmul(
                                                ps, wkm[:, kt, :],
                                                h1_fm[:, kt, qb * 512:(qb + 1) * 512],
                                                start=(kt == 0), stop=(kt == 7))
                                        ko = kout.tile([128, 512], F32R, tag="ko")
                                        nc.scalar.activation(out=ko, in_=ps, func=AF.Identity,
                                                             bias=bk_t[:, m:m + 1], scale=1.0)
                                        nc.gpsimd.dma_start(
                                            out=t["kfm"][m][:, qb * 512:(qb + 1) * 512],
                                            in_=ko)
                                for m in range(8):
                                    wqm = wstr.tile([128, 8, 128], F32R, tag="w")
                                    nc.sync.dma_start(
                                        out=wqm,
                                        in_=t["wq8"][m].rearrange("p (t n) -> p t n", t=8))
                                    for qb in range(2):
                                        ps = kps.tile([128, 512], F32, tag="ps")
                                        for kt in range(8):
                                            nc.tensor.matmul(
                                                ps, wqm[:, kt, :],
                                                h1_fm[:, kt, QOFF + qb * 512:QOFF + (qb + 1) * 512],
                                                start=(kt == 0), stop=(kt == 7))
                                        nc.scalar.activation(
                                            out=q_fm[:, m, qb * 512:(qb + 1) * 512],
                                            in_=ps, func=AF.Identity,
                                            bias=bq_t[:, m:m + 1], scale=1.0)

                        # ===== P2: attention (l-major) + early out-proj =====
                        with tc.tile_pool(name="wop", bufs=1) as wop:
                            wo_t = wop.tile([128, 8, D], BF16)
                            nc.sync.dma_start(
                                out=wo_t, in_=t["wod"].rearrange("p (t n) -> p t n", t=8))
                            bo_bc = wop.tile([128, D], F32, tag="bobc")
                            nc.sync.dma_start(out=bo_bc, in_=_bcast(t["bo"]))

                            with tc.tile_pool(name="mskp", bufs=1) as mskp, \
                                 tc.tile_pool(name="kvh", bufs=3) as kvh, \
                                 tc.tile_pool(name="ptp", bufs=6) as ptp, \
                                 tc.tile_pool(name="nrm", bufs=4) as nrm, \
                                 tc.tile_pool(name="oxp", bufs=2) as oxp, \
                                 tc.tile_pool(name="stps", bufs=2, space="PSUM") as stps, \
                                 tc.tile_pool(name="cxps", bufs=1, space="PSUM") as cxps, \
                                 tc.tile_pool(name="bcps", bufs=1, space="PSUM") as bcps, \
                                 tc.tile_pool(name="ops0", bufs=1, space="PSUM") as ops0:

                                mask_t = mskp.tile([128, 16, 512], BF16)
                                nc.sync.dma_start(
                                    out=mask_t,
                                    in_=t["mskd"].rearrange("p (m q) -> p m q", m=16))

                                def attn_l(l, filler=None):
                                    ext = 8 if l == 0 else 16
                                    for h2 in range(H // 2):
                                        if filler is not None:
                                            filler(h2)
                                        kh2 = kvh.tile([128, S], F32R, tag="kh",
                                                       name="kh2")
                                        nc.gpsimd.dma_start(out=kh2, in_=t["kfm"][h2])
                                        pcx = [None, None]
                                        for hh in range(2):
                                            pcx[hh] = cxps.tile([65, 512], F32,
                                                                tag=f"cx{hh}",
                                                                name=f"cx{hh}")
                                        for kt2 in range(ext // 2):
                                            kt0 = 2 * kt2
                                            masked = (l == 0) or (kt0 >= 8)
                                            for hh in range(2):
                                                h = 2 * h2 + hh
                                                pst = stps.tile([128, 2, 512], F32,
                                                                tag="st", name="pst")
                                                for j in range(2):
                                                    nc.tensor.matmul(
                                                        pst[:, j, :],
                                                        kh2[hh * 64:(hh + 1) * 64,
                                                            (kt0 + j) * 128:(kt0 + j + 1) * 128],
                                                        q_fm[hh * 64:(hh + 1) * 64, h2,
                                                             l * 512:(l + 1) * 512])
                                                pt = ptp.tile([128, 2, 512], BF16,
                                                              tag="pt", name="pt")
                                                nc.scalar.activation(out=pt, in_=pst,
                                                                     func=AF.Exp)
                                                if masked:
                                                    nc.vector.tensor_mul(
                                                        out=pt, in0=pt,
                                                        in1=mask_t[:, kt0:kt0 + 2, :])
                                                for j in range(2):
                                                    nc.tensor.matmul(
                                                        pcx[hh], vaug[:, kt0 + j, h, :],
                                                        pt[:, j, :],
                                                        start=(kt0 + j == 0),
                                                        stop=(kt0 + j == ext - 1))
                                        for hh in range(2):
                                            rec = nrm.tile([1, 512], F32R, tag="rec",
                                                           name="rec")
                                            with nc.allow_low_precision(
                                                    reason="softmax recip"):
                                                nc.vector.reciprocal(
                                                    out=rec, in_=pcx[hh][64:65, :])
                                            pbc = bcps.tile([64, 512], F32, tag="bc",
                                                            name="pbc")
                                            nc.tensor.matmul(pbc, ones64, rec)
                                            bcs = nrm.tile([64, 512], F32, tag="bcs",
                                                           name="bcs")
                                            nc.vector.tensor_copy(out=bcs, in_=pbc)
                                            nc.vector.tensor_mul(
                                                out=ctx_fm[hh * 64:(hh + 1) * 64, h2,
                                                           l * 512:(l + 1) * 512],
                                                in0=pcx[hh][0:64, :], in1=bcs)

                                attn_l(0)

                                # out-proj for own token tiles 0..3 (l=0 ctx columns
                                # complete), emitted one (tt, n) half-group per h2
                                # iteration of l=1 to fill PE's exp-wait gaps
                                def op_filler(h2):
                                    tt, n = h2 // 2, h2 % 2
                                    if n == 0:
                                        x_t = oxp.tile([128, D], F32, tag="ox",
                                                       name="x_t")
                                        nc.sync.dma_start(
                                            out=x_t,
                                            in_=t["xb"][QOFF + tt * 128:
                                                        QOFF + (tt + 1) * 128, :])
                                        op_filler.x = x_t
                                    x_t = op_filler.x
                                    po = ops0.tile([128, 512], F32, tag="po0",
                                                   name="po")
                                    for kt in range(8):
                                        nc.tensor.matmul(
                                            po,
                                            ctx_fm[:, kt, tt * 128:(tt + 1) * 128],
                                            wo_t[:, kt, n * 512:(n + 1) * 512],
                                            start=(kt == 0), stop=(kt == 7))
                                    yh = oxp.tile([128, 512], F32, tag="oy", name="yh")
                                    nc.vector.tensor_add(
                                        out=yh, in0=po,
                                        in1=bo_bc[:, n * 512:(n + 1) * 512])
                                    nc.vector.tensor_add(
                                        out=yh, in0=yh,
                                        in1=x_t[:, n * 512:(n + 1) * 512])
                                    nc.gpsimd.dma_start(
                                        out=t["y_d"][tt * 128:(tt + 1) * 128,
                                                     n * 512:(n + 1) * 512],
                                        in_=yh)

                                attn_l(1, filler=op_filler)

                # ===== P3 + P4 interleaved =====
                with tc.tile_pool(name="wop2", bufs=1) as wop2, \
                     tc.tile_pool(name="p3wrk", bufs=2) as wrk3, \
                     tc.tile_pool(name="p3stat", bufs=4) as stat3, \
                     tc.tile_pool(name="p3ps", bufs=1, space="PSUM") as ops, \
                     tc.tile_pool(name="p3psT", bufs=1, space="PSUM") as psT3, \
                     tc.tile_pool(name="w2p", bufs=1) as w2p, \
                     tc.tile_pool(name="w1str", bufs=3) as w1str, \
                     tc.tile_pool(name="acts", bufs=1) as acts, \
                     tc.tile_pool(name="p4out", bufs=2) as out4, \
                     tc.tile_pool(name="p4ps1", bufs=2, space="PSUM") as ps41, \
                     tc.tile_pool(name="p4ps2", bufs=2, space="PSUM") as ps42:
                    wo2_t = wop2.tile([128, 8, D], BF16)
                    nc.sync.dma_start(out=wo2_t,
                                      in_=t["wod"].rearrange("p (t n) -> p t n", t=8))
                    bo2_bc = wop2.tile([128, D], F32, tag="bobc2")
                    nc.sync.dma_start(out=bo2_bc, in_=_bcast(t["bo"]))
                    w2_t = w2p.tile([128, 32, D], BF16)
                    nc.gpsimd.dma_start(out=w2_t,
                                        in_=t["w2d"].rearrange("p (t n) -> p t n", t=32))

                    def ln2_to_h2(tt, yt):
                        h2_t = wrk3.tile([128, D], F32, tag="h2", name="h2_t")
                        _ln_normalize(nc, stat3, eps_t, yt, h2_t, "b")
                        for dp in range(4):
                            pt = psT3.tile([128, 2, 128], F32, tag="pt", name="pt")
                            for j in range(2):
                                nc.tensor.transpose(
                                    pt[:, j, :],
                                    h2_t[:, (2 * dp + j) * 128:(2 * dp + j + 1) * 128],
                                    ident)
                            dst = h2sb[:, 2 * dp:2 * dp + 2, tt * 128:(tt + 1) * 128]
                            if dp % 2 == 0:
                                nc.vector.tensor_copy(out=dst, in_=pt)
                            else:
                                nc.scalar.copy(out=dst, in_=pt)

                    def p3_lite(tt):
                        yl = wrk3.tile([128, D], F32, tag="y", name="yl")
                        nc.gpsimd.dma_start(
                            out=yl, in_=t["y_d"][tt * 128:(tt + 1) * 128, :])
                        ln2_to_h2(tt, yl)

                    def p3_tile(tt):
                        po = ops.tile([128, D], F32, tag="po", name="po")
                        for n in range(2):
                            for kt in range(8):
                                nc.tensor.matmul(po[:, n * 512:(n + 1) * 512],
                                                 ctx_fm[:, kt, tt * 128:(tt + 1) * 128],
                                                 wo2_t[:, kt, n * 512:(n + 1) * 512],
                                                 start=(kt == 0), stop=(kt == 7))
                        x_t = wrk3.tile([128, D], F32, tag="x", name="x_t")
                        nc.sync.dma_start(
                            out=x_t, in_=t["xb"][QOFF + tt * 128:QOFF + (tt + 1) * 128, :])
                        yt = wrk3.tile([128, D], F32, tag="y", name="yt")
                        nc.vector.tensor_add(out=yt, in0=po, in1=bo2_bc)
                        nc.vector.tensor_add(out=yt, in0=yt, in1=x_t)
                        nc.gpsimd.dma_start(out=t["y_d"][tt * 128:(tt + 1) * 128, :],
                                            in_=yt)
                        ln2_to_h2(tt, yt)

                    def fc1_chunk(qc):
                        a_qc = acts.tile([128, 32, 512], BF16, tag="a", name="a_qc")
                        for f in range(32):
                            w1f = w1str.tile([128, 8, 128], BF16, tag="w1", name="w1f")
                            nc.sync.dma_start(
                                out=w1f,
                                in_=t["w1t"][f].rearrange("p (t n) -> p t n", t=8))
                            ps = ps41.tile([128, 512], F32, tag="ps", name="ps")
                            for kt in range(8):
                                nc.tensor.matmul(ps, w1f[:, kt, :],
                                                 h2sb[:, kt, qc * 512:(qc + 1) * 512],
                                                 start=(kt == 0), stop=(kt == 7))
                            nc.scalar.activation(out=a_qc[:, f, :], in_=ps, func=AF.Silu,
                                                 scale=1.702, bias=b1s_t[:, f:f + 1])
                        return a_qc

                    def fc2_chunk(qc, a_qc):
                        for t4 in range(4):
                            yb = out4.tile([128, D], F32, tag="yb", name="yb")
                            nc.gpsimd.dma_start(
                                out=yb,
                                in_=t["y_d"][qc * 512 + t4 * 128:
                                             qc * 512 + (t4 + 1) * 128, :])
                            for n in range(2):
                                py = ps42.tile([128, 512], F32, tag="py", name="py")
                                for kt in range(32):
                                    nc.tensor.matmul(
                                        py,
                                        a_qc[:, kt, t4 * 128:(t4 + 1) * 128],
                                        w2_t[:, kt, n * 512:(n + 1) * 512],
                                        start=(kt == 0), stop=(kt == 31))
                                ot = out4.tile([128, 512], F32, tag="ot", name="ot")
                                nc.vector.tensor_add(out=ot, in0=py,
                                                     in1=b2_bc[:, n * 512:(n + 1) * 512])
                                nc.vector.tensor_add(out=ot, in0=ot,
                                                     in1=yb[:, n * 512:(n + 1) * 512])
                                nc.gpsimd.dma_start(
                                    out=t["yo"][qc * 512 + t4 * 128:
                                                qc * 512 + (t4 + 1) * 128,
                                                n * 512:(n + 1) * 512],
                                    in_=ot)

                    for tt in range(4):
                        p3_lite(tt)
                    a0 = fc1_chunk(0)
                    for tt in range(4, 8):
                        p3_tile(tt)
                    fc2_chunk(0, a0)
                    a1 = fc1_chunk(1)
                    fc2_chunk(1, a1)


def _perms():
    g0 = np.concatenate([np.arange(512, 1024), np.arange(0, 512),
                         np.arange(1536, 2048), np.arange(1024, 1536)])
    g1 = np.arange(S)
    return [g0, g1]


def _masks(perm):
    """[128, 16*512] bf16 pre-permuted mask: slots 0..7 = (l=0, kt), 8..15 = (l=1, kt)."""
    m = np.zeros((16, 128, 512), np.float32)
    for slot in range(16):
        l, kt = (0, slot) if slot < 8 else (1, slot)
        kg = perm[kt * 128:(kt + 1) * 128]
        qg = perm[QOFF + l * 512:QOFF + (l + 1) * 512]
        m[slot] = (kg[:, None] <= qg[None, :]).astype(np.float32)
    return np.ascontiguousarray(
        m.transpose(1, 0, 2).reshape(128, 16 * 512)).astype(ml_dtypes.bfloat16)


def _perm_w_mtiles(W, mt):
    """[Din, Dout] -> [mt, 128, Din//128 * (Dout//mt)]."""
    din, dout = W.shape
    n_sz = dout // mt
    A = W.reshape(din // 128, 128, mt, n_sz)
    return np.ascontiguousarray(A.transpose(2, 1, 0, 3).reshape(mt, 128, -1))


def _prep_consts(inputs):
    f = {k: np.asarray(v, np.float64) for k, v in inputs.items()}
    g1, b1 = f["ln1_g"], f["ln1_b"]
    g2, b2 = f["ln2_g"], f["ln2_b"]
    qs = 1.0 / np.sqrt(HD)
    wq = ((g1[:, None] * f["Wq"]) * qs).astype(np.float32)
    wk = (g1[:, None] * f["Wk"]).astype(np.float32)
    wv = (g1[:, None] * f["Wv"]).astype(np.float32)
    w1 = (g2[:, None] * f["W1"]).astype(np.float32)
    bf = ml_dtypes.bfloat16
    c = {}
    c["wq8"] = _perm_w_mtiles(wq, 8)
    c["wk8"] = _perm_w_mtiles(wk, 8)
    c["wvd"] = np.ascontiguousarray(wv.reshape(8, 128, 2, 512).transpose(1, 2, 0, 3))
    c["wod"] = np.ascontiguousarray(
        f["Wo"].astype(np.float32).reshape(8, 128, D).transpose(1, 0, 2)
        .reshape(128, 8 * D)).astype(bf)
    c["w1t"] = _perm_w_mtiles(w1, 32).astype(bf)
    c["w2d"] = np.ascontiguousarray(
        (f["W2"] / 1.702).astype(np.float32)
        .reshape(32, 128, D).transpose(1, 0, 2).reshape(128, 32 * D)).astype(bf)
    c["bqd"] = np.ascontiguousarray(
        ((b1 @ f["Wq"] + f["bq"]) * qs).astype(np.float32).reshape(8, 128).T)
    c["bkd"] = np.ascontiguousarray(
        (b1 @ f["Wk"] + f["bk"]).astype(np.float32).reshape(8, 128).T)
    c["b1d"] = np.ascontiguousarray(
        (1.702 * (b2 @ f["W1"] + f["b1"])).astype(np.float32).reshape(32, 128).T)
    c["bv"] = (b1 @ f["Wv"] + f["bv"]).astype(np.float32)
    c["bo"] = f["bo"].astype(np.float32)
    c["b2"] = f["b2"].astype(np.float32)
    return c


def kernel(**inputs):
    if "nc" not in _CACHE:
        _CACHE["nc"] = _build_program()
        _CACHE["perms"] = _perms()
        _CACHE["masks"] = [_masks(p) for p in _CACHE["perms"]]
    nc = _CACHE["nc"]
    perms, masks = _CACHE["perms"], _CACHE["masks"]

    x = np.asarray(inputs["x"], np.float32)
    c = _prep_consts(inputs)

    in_maps = []
    for core in range(NCORES):
        b, s = core // 2, core % 2
        m = dict(c)
        m["xb"] = np.ascontiguousarray(x[b][perms[s]])
        m["mskd"] = masks[s]
        in_maps.append(m)

    res = run_bass_kernel_spmd(nc, in_maps, core_ids=list(range(NCORES)))

    out = np.empty((B, S, D), np.float32)
    for core in range(NCORES):
        b, s = core // 2, core % 2
        out[b][perms[s][QOFF:QOFF + OWN]] = res.results[core]["yo"]
    return out



# revision 4
# speedup vs baseline: 1.0411x; 1.0411x over previous
"""CLIP encoder layer (LN -> causal MHA -> residual -> LN -> quickGELU MLP -> residual)
on 8 Trainium2 NeuronCores, SPMD via bass/Tile. v2.

Sharding: 8 shards = 4 batches x 2 query-groups. Core c handles batch c//2 with
parity s = c%2, owning 1024 query tokens in 4 phase-groups of 256. Each core
recomputes LN1 + K/V for the full sequence of its batch (no collectives).

4-phase causal schedule: phase p attends its 256 own queries against the first
EXT[p] = (4,8,12,16) key tiles of 128. A per-parity permutation of 256-token
blocks (pairs swapped) makes the local key order prefix-consistent for both
parities while own queries sit at uniform local offsets 256+512p. Only the last
4 key tiles of each phase are masked (diagonal + parity waste), via data.

Everything on the matmul path is bf16 (tolerance 2e-2 allows it): LN outputs
bf16, transposed token-major -> feature-major by the DMA XBAR transpose (14ns/
tile, zero engine cost) instead of PE transposes. K stays resident in SBUF (no
DRAM spill), y (residual stream) stays resident in SBUF. Softmax uses the
augmented-V ones-row trick for sums; the reciprocal is broadcast across
partitions by GPSIMD partition_broadcast (no PE/DVE broadcast work). Weights
are pre-permuted on the host; LN affines, q-scale and quickGELU's 1.702 are
folded into weights/biases.
"""

import sys

sys.path.insert(0, "/opt/trn_rl_repo")

import numpy as np
import ml_dtypes

import concourse.bass as bass
import concourse.mybir as mybir
import concourse.tile as tile
from concourse import bacc
from concourse.bass_utils import run_bass_kernel_spmd

B, S, D = 4, 2048, 1024
H, HD = 16, 64
NCORES = 8
EPS = 1e-5
OWN = 1024
EXT = [4, 8, 12, 16]
F32 = mybir.dt.float32
BF16 = mybir.dt.bfloat16
FP8 = mybir.dt.float8e4
ALU = mybir.AluOpType
AF = mybir.ActivationFunctionType

_CACHE = {}


def _bcast(ap1d, p=128):
    """[N] DRAM vector viewed as [p, N] with partition-step 0 (broadcast load)."""
    return bass.AP(tensor=ap1d.tensor, offset=ap1d.offset, ap=[[0, p]] + list(ap1d.ap))


def _build_program():
    nc = bacc.Bacc("TRN2", target_bir_lowering=False, debug=False,
                   num_devices=NCORES)

    t = {}
    t["xb"] = nc.dram_tensor("xb", [S, D], F32, kind="ExternalInput").ap()
    t["xh"] = nc.dram_tensor("xh", [S, D], BF16, kind="ExternalInput").ap()
    t["wq8"] = nc.dram_tensor("wq8", [8, 128, D], BF16, kind="ExternalInput").ap()
    t["wk8"] = nc.dram_tensor("wk8", [8, 128, D], BF16, kind="ExternalInput").ap()
    t["wvd"] = nc.dram_tensor("wvd", [128, 2, 8, 512], BF16, kind="ExternalInput").ap()
    t["wod"] = nc.dram_tensor("wod", [128, 8 * D], BF16, kind="ExternalInput").ap()
    t["w1t"] = nc.dram_tensor("w1t", [32, 128, D], BF16, kind="ExternalInput").ap()
    t["w2d"] = nc.dram_tensor("w2d", [128, 32 * D], BF16, kind="ExternalInput").ap()
    t["mskd"] = nc.dram_tensor("mskd", [128, 16 * 256], BF16, kind="ExternalInput").ap()
    t["bqd"] = nc.dram_tensor("bqd", [128, 8], F32, kind="ExternalInput").ap()
    t["bkd"] = nc.dram_tensor("bkd", [128, 8], F32, kind="ExternalInput").ap()
    t["b1d"] = nc.dram_tensor("b1d", [128, 32], F32, kind="ExternalInput").ap()
    for b_ in ("bo", "b2"):
        t[b_] = nc.dram_tensor(b_, [D], F32, kind="ExternalInput").ap()
    t["yo"] = nc.dram_tensor("yo", [OWN, D], F32, kind="ExternalOutput").ap()

    with tile.TileContext(nc) as tc:
        _body(nc, tc, t)
    nc.compile()
    return nc


def _ln_normalize(nc, stat, eps_t, x_t, out_t, tag, on_dve=False):
    """LayerNorm (affine folded into downstream weights): out = (x-mu)*rstd.
    Stats on DVE; the big normalize pass on ACT, or on DVE (tensor_scalar)
    when ACT is the busier engine (attention-phase LN2)."""
    st = stat.tile([128, 2, 6], F32, tag=f"{tag}st")
    for g in range(2):
        nc.vector.bn_stats(out=st[:, g, :], in_=x_t[:, g * 512:(g + 1) * 512])
    mv = stat.tile([128, 2], F32, tag=f"{tag}mv")
    nc.vector.bn_aggr(out=mv, in_=st)
    rstd = stat.tile([128, 1], F32, tag=f"{tag}rs")
    nc.scalar.activation(out=rstd, in_=mv[:, 1:2], func=AF.Sqrt, bias=eps_t, scale=1.0)
    nc.vector.reciprocal(out=rstd, in_=rstd)
    nmu = stat.tile([128, 1], F32, tag=f"{tag}nm")
    nc.vector.tensor_scalar(out=nmu, in0=mv[:, 0:1], scalar1=rstd, scalar2=-1.0,
                            op0=ALU.mult, op1=ALU.mult)
    if on_dve:
        nc.vector.tensor_scalar(out=out_t, in0=x_t, scalar1=rstd, scalar2=nmu,
                                op0=ALU.mult, op1=ALU.add)
    else:
        nc.scalar.activation(out=out_t, in_=x_t, func=AF.Identity, bias=nmu,
                             scale=rstd)


def _body(nc, tc, t):
    with tc.tile_pool(name="const", bufs=1) as const:
        eps_t = const.tile([128, 1], F32)
        nc.vector.memset(eps_t, EPS)
        bq_t = const.tile([128, 8], F32)
        bk_t = const.tile([128, 8], F32)
        b1s_t = const.tile([128, 32], F32)

        with tc.tile_pool(name="ysbp", bufs=1) as ysbp, \
             tc.tile_pool(name="h2p", bufs=1) as h2p, \
             tc.tile_pool(name="ln2s", bufs=4) as stat2, \
             tc.tile_pool(name="ln2w", bufs=2) as wrk2:
            y_sb = ysbp.tile([128, 8, D], F32)
            h2sb = [h2p.tile([128, 8, 256], BF16, tag=f"h2sb{c}",
                             name=f"h2sb{c}") for c in range(4)]

            with tc.tile_pool(name="vaugp", bufs=1) as vaugp, \
                 tc.tile_pool(name="qfmp", bufs=1) as qfmp, \
                 tc.tile_pool(name="ksbp", bufs=1) as ksbp:
                vaug = vaugp.tile([128, 16, H, 65], BF16)
                q_fm = qfmp.tile([128, 8, OWN], BF16)
                ksb = ksbp.tile([128, 8, S], BF16)

                nc.vector.memset(vaug[:, :, :, 64:65], 1.0)

                # ===== P0+P1 fused: LN1 / V / K / Q =====
                with tc.tile_pool(name="p01h1", bufs=1) as h1p, \
                     tc.tile_pool(name="p01x", bufs=6) as xpool, \
                     tc.tile_pool(name="p01s", bufs=6) as stat, \
                     tc.tile_pool(name="p01wv", bufs=2) as wvp, \
                     tc.tile_pool(name="p01wst", bufs=3) as wstr, \
                     tc.tile_pool(name="p01vps", bufs=2, space="PSUM") as vps, \
                     tc.tile_pool(name="p01kps", bufs=2, space="PSUM") as kps, \
                     tc.tile_pool(name="p01qps", bufs=2, space="PSUM") as qps:
                    h1_fm = h1p.tile([128, 8, S], BF16)

                    def ln1_tile(tt, on_dve):
                        x_t = xpool.tile([128, D], BF16, tag="x", name="x_t")
                        nc.gpsimd.dma_start(
                            out=x_t, in_=t["xh"][tt * 128:(tt + 1) * 128, :])
                        _ln_normalize(nc, stat, eps_t, x_t, x_t, "a",
                                      on_dve=on_dve)
                        nc.sync.dma_start_transpose(
                            out=h1_fm[:, :, tt * 128:(tt + 1) * 128], in_=x_t)

                    def load_wv(fb):
                        wvh = wvp.tile([128, 8, 512], BF16, tag="wv", name="wvh")
                        nc.gpsimd.dma_start(out=wvh, in_=t["wvd"][:, fb, :, :])
                        return wvh

                    def v_tile(wvh, fb, tt, on_act):
                        ps = vps.tile([128, 512], F32, tag="ps", name="vps")
                        for kt in range(8):
                            nc.tensor.matmul(
                                ps, h1_fm[:, kt, tt * 128:(tt + 1) * 128],
                                wvh[:, kt, :],
                                start=(kt == 0), stop=(kt == 7))
                        dst = vaug[:, tt, fb * 8:(fb + 1) * 8, 0:64]
                        if on_act:
                            nc.scalar.copy(
                                out=dst, in_=ps.rearrange("p (h f) -> p h f", h=8))
                        else:
                            nc.vector.tensor_copy(
                                out=dst, in_=ps.rearrange("p (h f) -> p h f", h=8))

                    def k_pass(qbs, on_act):
                        for m in range(8):
                            wkm = wstr.tile([128, 8, 128], BF16, tag="w", name="wkm")
                            nc.sync.dma_start(
                                out=wkm,
                                in_=t["wk8"][m].rearrange("p (t n) -> p t n", t=8))
                            for qb in (qbs if isinstance(qbs, list) else [qbs]):
                                ps = kps.tile([128, 512], F32, tag="ps", name="kps")
                                for kt in range(8):
                                    nc.tensor.matmul(
                                        ps, wkm[:, kt, :],
                                        h1_fm[:, kt, qb * 512:(qb + 1) * 512],
                                        start=(kt == 0), stop=(kt == 7))
                                dst = ksb[:, m, qb * 512:(qb + 1) * 512]
                                if on_act:
                                    nc.scalar.activation(
                                        out=dst, in_=ps, func=AF.Identity,
                                        bias=bk_t[:, m:m + 1], scale=1.0)
                                else:
                                    nc.vector.tensor_scalar_add(
                                        out=dst, in0=ps,
                                        scalar1=bk_t[:, m:m + 1])

                    def q_pass(phases, on_act):
                        for m in range(8):
                            wqm = wstr.tile([128, 8, 128], BF16, tag="w", name="wqm")
                            nc.sync.dma_start(
                                out=wqm,
                                in_=t["wq8"][m].rearrange("p (t n) -> p t n", t=8))
                            for p in phases:
                                ps = qps.tile([128, 256], F32, tag="ps", name="qps")
                                for kt in range(8):
                                    nc.tensor.matmul(
                                        ps, wqm[:, kt, :],
                                        h1_fm[:, kt, 256 + 512 * p:512 + 512 * p],
                                        start=(kt == 0), stop=(kt == 7))
                                dst = q_fm[:, m, p * 256:(p + 1) * 256]
                                if on_act:
                                    nc.scalar.activation(
                                        out=dst, in_=ps, func=AF.Identity,
                                        bias=bq_t[:, m:m + 1], scale=1.0)
                                else:
                                    nc.vector.tensor_scalar_add(
                                        out=dst, in0=ps,
                                        scalar1=bq_t[:, m:m + 1])

                    for half in range(2):
                        t0 = 8 * half
                        on_act = (half == 0)
                        ln1_tile(t0, not on_act)
                        wvh0 = load_wv(0)
                        ln1_tile(t0 + 1, not on_act)
                        ln1_tile(t0 + 2, not on_act)
                        ln1_tile(t0 + 3, not on_act)
                        if half == 0:
                            nc.gpsimd.dma_start(out=bq_t, in_=t["bqd"])
                            nc.gpsimd.dma_start(out=bk_t, in_=t["bkd"])
                        for i in range(4):
                            ln1_tile(t0 + 4 + i, not on_act)
                            v_tile(wvh0, 0, t0 + i, on_act)
                        k_pass(2 * half, on_act)
                        for i in range(4, 8):
                            v_tile(wvh0, 0, t0 + i, on_act)
                        wvh1 = load_wv(1)
                        for i in range(8):
                            v_tile(wvh1, 1, t0 + i, on_act)
                        k_pass(2 * half + 1, on_act)
                        q_pass([2 * half, 2 * half + 1], on_act)
                    for p in range(4):
                        nc.sync.dma_start(
                            out=y_sb[:, 2 * p:2 * p + 2, :],
                            in_=bass.AP(tensor=t["xb"].tensor,
                                        offset=(256 + 512 * p) * D,
                                        ap=[[D, 128], [128 * D, 2], [1, D]]))

                # ===== P2: attention, 4 causal phases + fillers =====
                with tc.tile_pool(name="ctxp", bufs=1) as ctxp, \
                     tc.tile_pool(name="wop", bufs=1) as wop, \
                     tc.tile_pool(name="amsk", bufs=1) as mskp, \
                     tc.tile_pool(name="apt", bufs=3) as ptp, \
                     tc.tile_pool(name="anrm", bufs=4) as nrm, \
                     tc.tile_pool(name="aox", bufs=2) as oxp, \
                     tc.tile_pool(name="astps", bufs=2, space="PSUM") as stps, \
                     tc.tile_pool(name="acxps", bufs=1, space="PSUM") as cxps, \
                     tc.tile_pool(name="aops", bufs=1, space="PSUM") as opps:
                    ctx_fm = ctxp.tile([128, 8, OWN], BF16)
                    wo_t = wop.tile([128, 8, D], BF16)
                    bo_bc = wop.tile([128, D], F32, tag="bo", name="bo_bc")
                    mask_t = mskp.tile([128, 16, 256], BF16)
                    nc.gpsimd.dma_start(
                        out=mask_t,
                        in_=t["mskd"].rearrange("p (m q) -> p m q", m=16))
                    nc.gpsimd.dma_start(
                        out=wo_t, in_=t["wod"].rearrange("p (t n) -> p t n", t=8))
                    nc.gpsimd.dma_start(out=bo_bc, in_=_bcast(t["bo"]))

                    def ln2_tile(j):
                        h2_t = wrk2.tile([128, D], BF16, tag="h2t", name="h2t")
                        _ln_normalize(nc, stat2, eps_t, y_sb[:, j, :], h2_t, "b",
                                      on_dve=(j >= 2))
                        nc.sync.dma_start_transpose(
                            out=h2sb[j // 2][:, :, (j % 2) * 128:
                                             (j % 2) * 128 + 128], in_=h2_t)

                    def op_tile(j, n):
                        po = opps.tile([128, 512], F32, tag="po", name="po")
                        for kt in range(8):
                            nc.tensor.matmul(
                                po, ctx_fm[:, kt, j * 128:(j + 1) * 128],
                                wo_t[:, kt, n * 512:(n + 1) * 512],
                                start=(kt == 0), stop=(kt == 7))
                        yh = oxp.tile([128, 512], F32, tag="oy", name="yh")
                        nc.vector.tensor_add(
                            out=yh, in0=po, in1=bo_bc[:, n * 512:(n + 1) * 512])
                        nc.vector.tensor_add(
                            out=y_sb[:, j, n * 512:(n + 1) * 512],
                            in0=y_sb[:, j, n * 512:(n + 1) * 512], in1=yh)

                    pending = [None]  # deferred softmax finalize (pcx, p, h2)

                    def finalize_ctx():
                        if pending[0] is None:
                            return
                        pcx, fp, fh2 = pending[0]
                        pending[0] = None
                        for hh in range(2):
                            rec = nrm.tile([1, 256], F32, tag="rec", name="rec")
                            nc.vector.reciprocal(out=rec, in_=pcx[hh][64:65, :])
                            pb = nrm.tile([64, 256], F32, tag="pb", name="pb")
                            nc.gpsimd.partition_broadcast(pb, rec)
                            nc.vector.tensor_mul(
                                out=ctx_fm[hh * 64:(hh + 1) * 64, fh2,
                                           fp * 256:(fp + 1) * 256],
                                in0=pcx[hh][0:64, :], in1=pb)

                    def attn_phase(p, filler):
                        ext = EXT[p]
                        chunks = ext // 4
                        # masked chunk (diagonal, extra mask hop) first in the
                        # accumulation; short unmasked chain closes the group
                        order = [chunks - 1] + list(range(chunks - 1))
                        for h2 in range(8):
                            pcx = [cxps.tile([65, 256], F32, tag=f"cx{hh}",
                                             name=f"cx{hh}") for hh in range(2)]
                            pts = {}

                            def sc(hh, c):
                                pst = stps.tile([128, 4, 256], F32, tag="st",
                                                name="pst")
                                for j in range(4):
                                    kt = 4 * c + j
                                    nc.tensor.matmul(
                                        pst[:, j, :],
                                        ksb[hh * 64:(hh + 1) * 64, h2,
                                            kt * 128:(kt + 1) * 128],
                                        q_fm[hh * 64:(hh + 1) * 64, h2,
                                             p * 256:(p + 1) * 256])
                                pt = ptp.tile([128, 4, 256], BF16, tag="pt",
                                              name="pt")
                                nc.scalar.activation(out=pt, in_=pst, func=AF.Exp)
                                if c == chunks - 1:
                                    nc.vector.tensor_mul(
                                        out=pt, in0=pt,
                                        in1=mask_t[:, 4 * p:4 * p + 4, :])
                                pts[(hh, c)] = pt

                            def av(hh, ci):
                                h = 2 * h2 + hh
                                c = order[ci]
                                pt = pts.pop((hh, c))
                                for j in range(4):
                                    kt = 4 * c + j
                                    nc.tensor.matmul(
                                        pcx[hh], vaug[:, kt, h, :], pt[:, j, :],
                                        start=(ci == 0 and j == 0),
                                        stop=(ci == chunks - 1 and j == 3))

                            sc(0, order[0])
                            sc(1, order[0])
                            finalize_ctx()
                            filler(h2)
                            for ci in range(1, chunks):
                                av(0, ci - 1)
                                sc(0, order[ci])
                                av(1, ci - 1)
                                sc(1, order[ci])
                            av(0, chunks - 1)
                            av(1, chunks - 1)
                            pending[0] = (pcx, p, h2)

                    def mk_filler(g):
                        if g < 0:
                            return lambda h2: None
                        thunks = [
                            lambda: op_tile(2 * g, 0),
                            lambda: op_tile(2 * g, 1),
                            lambda: ln2_tile(2 * g),
                            lambda: op_tile(2 * g + 1, 0),
                            lambda: op_tile(2 * g + 1, 1),
                            lambda: ln2_tile(2 * g + 1),
                            None, None,
                        ]

                        def filler(h2):
                            th = thunks[h2]
                            if th is not None:
                                th()
                        return filler

                    for p in range(4):
                        attn_phase(p, mk_filler(p - 1))
                    finalize_ctx()
                    # group 3 out-proj + LN2 (ctx/wo die with this scope)
                    for j in (6, 7):
                        op_tile(j, 0)
                        op_tile(j, 1)
                        ln2_tile(j)

            # ===== P4: MLP (single-pass w1 stream, resident w2) =====
            with tc.tile_pool(name="mw2", bufs=1) as w2p, \
                 tc.tile_pool(name="mw1", bufs=4) as w1str, \
                 tc.tile_pool(name="macts", bufs=1) as acts, \
                 tc.tile_pool(name="mout", bufs=3) as out4, \
                 tc.tile_pool(name="mps1", bufs=3, space="PSUM") as ps41, \
                 tc.tile_pool(name="mps2", bufs=3, space="PSUM") as ps42:
                w2_t = w2p.tile([128, 32, D], BF16)
                b2_bc = w2p.tile([128, D], F32, tag="b2", name="b2_bc")
                nc.gpsimd.dma_start(out=b1s_t, in_=t["b1d"])
                nc.gpsimd.dma_start(out=b2_bc, in_=_bcast(t["b2"]))
                a_c = [acts.tile([128, 32, 256], BF16, tag=f"a{c}", name=f"a{c}")
                       for c in range(4)]

                def fc1_pass(cs):
                    for f in range(32):
                        w1f = w1str.tile([128, 8, 128], BF16, tag="w1",
                                         name="w1f")
                        nc.gpsimd.dma_start(
                            out=w1f,
                            in_=t["w1t"][f].rearrange("p (t n) -> p t n", t=8))
                        for c in cs:
                            ps = ps41.tile([128, 256], F32, tag="ps", name="ps")
                            for kt in range(8):
                                nc.tensor.matmul(
                                    ps, w1f[:, kt, :],
                                    h2sb[c][:, kt, :],
                                    start=(kt == 0), stop=(kt == 7))
                            nc.scalar.activation(out=a_c[c][:, f, :], in_=ps,
                                                 func=AF.Silu, scale=1.702,
                                                 bias=b1s_t[:, f:f + 1])
                        if f % 6 == 0 and f > 0 and f <= 24 and cs[0] == 0:
                            q = f // 6 - 1
                            nc.sync.dma_start(
                                out=w2_t[:, 8 * q:8 * (q + 1), :],
                                in_=t["w2d"][:, q * 8192:(q + 1) * 8192]
                                .rearrange("p (a c) -> p a c", a=8))

                def fc2_chunk(c):
                    for t2 in range(2):
                        j = 2 * c + t2
                        for n in range(2):
                            py = ps42.tile([128, 512], F32, tag="py", name="py")
                            for kt in range(32):
                                nc.tensor.matmul(
                                    py, a_c[c][:, kt, t2 * 128:(t2 + 1) * 128],
                                    w2_t[:, kt, n * 512:(n + 1) * 512],
                                    start=(kt == 0), stop=(kt == 31))
                            ot = out4.tile([128, 512], F32, tag="ot", name="ot")
                            nc.vector.tensor_add(
                                out=ot, in0=py,
                                in1=b2_bc[:, n * 512:(n + 1) * 512])
                            nc.vector.tensor_add(
                                out=ot, in0=ot,
                                in1=y_sb[:, j, n * 512:(n + 1) * 512])
                            nc.scalar.dma_start(
                                out=t["yo"][j * 128:(j + 1) * 128,
                                            n * 512:(n + 1) * 512],
                                in_=ot)

                fc1_pass([0, 1, 2])
                fc2_chunk(0)
                fc1_pass([3])
                fc2_chunk(1)
                fc2_chunk(2)
                fc2_chunk(3)


def _perms():
    g = [np.arange(256 * i, 256 * (i + 1)) for i in range(8)]
    order = [[1, 0, 2, 3, 5, 4, 6, 7], [0, 1, 3, 2, 4, 5, 7, 6]]
    return [np.concatenate([g[i] for i in o]) for o in order]


def _masks(perm):
    """[128, 16*256] bf16: slot 4p+i covers key tile EXT[p]-4+i of phase p."""
    m = np.zeros((16, 128, 256), np.float32)
    for p in range(4):
        qg = perm[256 + 512 * p:512 + 512 * p]
        for i in range(4):
            kt = EXT[p] - 4 + i
            kg = perm[kt * 128:(kt + 1) * 128]
            m[4 * p + i] = (kg[:, None] <= qg[None, :]).astype(np.float32)
    return np.ascontiguousarray(
        m.transpose(1, 0, 2).reshape(128, 16 * 256)).astype(ml_dtypes.bfloat16)


def _perm_w_mtiles(W, mt):
    """[Din, Dout] -> [mt, 128, Din//128 * (Dout//mt)]."""
    din, dout = W.shape
    n_sz = dout // mt
    A = W.reshape(din // 128, 128, mt, n_sz)
    return np.ascontiguousarray(A.transpose(2, 1, 0, 3).reshape(mt, 128, -1))


def _prep_consts(inputs):
    f = {k: np.asarray(v, np.float64) for k, v in inputs.items()}
    g1, b1 = f["ln1_g"], f["ln1_b"]
    g2, b2 = f["ln2_g"], f["ln2_b"]
    qs = 1.0 / np.sqrt(HD)
    wq = ((g1[:, None] * f["Wq"]) * qs).astype(np.float32)
    wk = (g1[:, None] * f["Wk"]).astype(np.float32)
    wv = (g1[:, None] * f["Wv"]).astype(np.float32)
    w1 = (g2[:, None] * f["W1"]).astype(np.float32)
    bf = ml_dtypes.bfloat16
    c = {}
    c["wq8"] = _perm_w_mtiles(wq, 8).astype(bf)
    c["wk8"] = _perm_w_mtiles(wk, 8).astype(bf)
    c["wvd"] = np.ascontiguousarray(
        wv.reshape(8, 128, 2, 512).transpose(1, 2, 0, 3)).astype(bf)
    c["wod"] = np.ascontiguousarray(
        f["Wo"].astype(np.float32).reshape(8, 128, D).transpose(1, 0, 2)
        .reshape(128, 8 * D)).astype(bf)
    c["w1t"] = _perm_w_mtiles(w1, 32).astype(bf)
    c["w2d"] = np.ascontiguousarray(
        (f["W2"] / 1.702).astype(np.float32)
        .reshape(32, 128, D).transpose(1, 0, 2).reshape(128, 32 * D)).astype(bf)
    c["bqd"] = np.ascontiguousarray(
        ((b1 @ f["Wq"] + f["bq"]) * qs).astype(np.float32).reshape(8, 128).T)
    c["bkd"] = np.ascontiguousarray(
        (b1 @ f["Wk"] + f["bk"]).astype(np.float32).reshape(8, 128).T)
    c["b1d"] = np.ascontiguousarray(
        (1.702 * (b2 @ f["W1"] + f["b1"])).astype(np.float32).reshape(32, 128).T)
    bv_eff = b1 @ f["Wv"] + f["bv"]
    c["bo"] = (bv_eff @ f["Wo"] + f["bo"]).astype(np.float32)
    c["b2"] = f["b2"].astype(np.float32)
    return c


def kernel(**inputs):
    if "nc" not in _CACHE:
        _CACHE["nc"] = _build_program()
        _CACHE["perms"] = _perms()
        _CACHE["masks"] = [_masks(p) for p in _CACHE["perms"]]
    nc = _CACHE["nc"]
    perms, masks = _CACHE["perms"], _CACHE["masks"]

    x = np.asarray(inputs["x"], np.float32)
    c = _prep_consts(inputs)

    in_maps = []
    for core in range(NCORES):
        b, s = core // 2, core % 2
        m = dict(c)
        m["xb"] = np.ascontiguousarray(x[b][perms[s]])
        m["xh"] = m["xb"].astype(ml_dtypes.bfloat16)
        m["mskd"] = masks[s]
        in_maps.append(m)

    res = run_bass_kernel_spmd(nc, in_maps, core_ids=list(range(NCORES)))

    own_local = np.concatenate(
        [np.arange(256 + 512 * p, 512 + 512 * p) for p in range(4)])
    out = np.empty((B, S, D), np.float32)
    for core in range(NCORES):
        b, s = core // 2, core % 2
        out[b][perms[s][own_local]] = res.results[core]["yo"]
    return out


# revision 5
# speedup vs baseline: 1.0465x; 1.0052x over previous
"""CLIP encoder layer (LN -> causal MHA -> residual -> LN -> quickGELU MLP -> residual)
on 8 Trainium2 NeuronCores, SPMD via bass/Tile. v2.

Sharding: 8 shards = 4 batches x 2 query-groups. Core c handles batch c//2 with
parity s = c%2, owning 1024 query tokens in 4 phase-groups of 256. Each core
recomputes LN1 + K/V for the full sequence of its batch (no collectives).

4-phase causal schedule: phase p attends its 256 own queries against the first
EXT[p] = (4,8,12,16) key tiles of 128. A per-parity permutation of 256-token
blocks (pairs swapped) makes the local key order prefix-consistent for both
parities while own queries sit at uniform local offsets 256+512p. Only the last
4 key tiles of each phase are masked (diagonal + parity waste), via data.

Everything on the matmul path is bf16 (tolerance 2e-2 allows it): LN outputs
bf16, transposed token-major -> feature-major by the DMA XBAR transpose (14ns/
tile, zero engine cost) instead of PE transposes. K stays resident in SBUF (no
DRAM spill), y (residual stream) stays resident in SBUF. Softmax uses the
augmented-V ones-row trick for sums; the reciprocal is broadcast across
partitions by GPSIMD partition_broadcast (no PE/DVE broadcast work). Weights
are pre-permuted on the host; LN affines, q-scale and quickGELU's 1.702 are
folded into weights/biases.
"""

import sys

sys.path.insert(0, "/opt/trn_rl_repo")

import numpy as np
import ml_dtypes

import concourse.bass as bass
import concourse.mybir as mybir
import concourse.tile as tile
from concourse import bacc
from concourse.bass_utils import run_bass_kernel_spmd

B, S, D = 4, 2048, 1024
H, HD = 16, 64
NCORES = 8
EPS = 1e-5
OWN = 1024
EXT = [4, 8, 12, 16]
F32 = mybir.dt.float32
BF16 = mybir.dt.bfloat16
FP8 = mybir.dt.float8e4
ALU = mybir.AluOpType
AF = mybir.ActivationFunctionType

_CACHE = {}


def _bcast(ap1d, p=128):
    """[N] DRAM vector viewed as [p, N] with partition-step 0 (broadcast load)."""
    return bass.AP(tensor=ap1d.tensor, offset=ap1d.offset, ap=[[0, p]] + list(ap1d.ap))


def _build_program():
    nc = bacc.Bacc("TRN2", target_bir_lowering=False, debug=False,
                   num_devices=NCORES)

    t = {}
    t["xb"] = nc.dram_tensor("xb", [S, D], F32, kind="ExternalInput").ap()
    t["xh"] = nc.dram_tensor("xh", [S, D], BF16, kind="ExternalInput").ap()
    t["wq8"] = nc.dram_tensor("wq8", [8, 128, D], BF16, kind="ExternalInput").ap()
    t["wk8"] = nc.dram_tensor("wk8", [8, 128, D], BF16, kind="ExternalInput").ap()
    t["wvd"] = nc.dram_tensor("wvd", [128, 2, 8, 512], BF16, kind="ExternalInput").ap()
    t["wod"] = nc.dram_tensor("wod", [128, 8 * D], BF16, kind="ExternalInput").ap()
    t["w1t"] = nc.dram_tensor("w1t", [32, 128, D], BF16, kind="ExternalInput").ap()
    t["w2d"] = nc.dram_tensor("w2d", [128, 32 * D], BF16, kind="ExternalInput").ap()
    t["mskd"] = nc.dram_tensor("mskd", [128, 16 * 256], BF16, kind="ExternalInput").ap()
    t["bqd"] = nc.dram_tensor("bqd", [128, 8], F32, kind="ExternalInput").ap()
    t["bkd"] = nc.dram_tensor("bkd", [128, 8], F32, kind="ExternalInput").ap()
    t["b1d"] = nc.dram_tensor("b1d", [128, 32], F32, kind="ExternalInput").ap()
    for b_ in ("bo", "b2"):
        t[b_] = nc.dram_tensor(b_, [D], F32, kind="ExternalInput").ap()
    t["yo"] = nc.dram_tensor("yo", [OWN, D], F32, kind="ExternalOutput").ap()

    with tile.TileContext(nc) as tc:
        _body(nc, tc, t)
    nc.compile()
    return nc


def _ln_normalize(nc, stat, eps_t, x_t, out_t, tag, on_dve=False):
    """LayerNorm (affine folded into downstream weights): out = (x-mu)*rstd.
    Stats on DVE; the big normalize pass on ACT, or on DVE (tensor_scalar)
    when ACT is the busier engine (attention-phase LN2)."""
    st = stat.tile([128, 2, 6], F32, tag=f"{tag}st")
    for g in range(2):
        nc.vector.bn_stats(out=st[:, g, :], in_=x_t[:, g * 512:(g + 1) * 512])
    mv = stat.tile([128, 2], F32, tag=f"{tag}mv")
    nc.vector.bn_aggr(out=mv, in_=st)
    rstd = stat.tile([128, 1], F32, tag=f"{tag}rs")
    nc.scalar.activation(out=rstd, in_=mv[:, 1:2], func=AF.Sqrt, bias=eps_t, scale=1.0)
    nc.vector.reciprocal(out=rstd, in_=rstd)
    nmu = stat.tile([128, 1], F32, tag=f"{tag}nm")
    nc.vector.tensor_scalar(out=nmu, in0=mv[:, 0:1], scalar1=rstd, scalar2=-1.0,
                            op0=ALU.mult, op1=ALU.mult)
    if on_dve:
        nc.vector.tensor_scalar(out=out_t, in0=x_t, scalar1=rstd, scalar2=nmu,
                                op0=ALU.mult, op1=ALU.add)
    else:
        nc.scalar.activation(out=out_t, in_=x_t, func=AF.Identity, bias=nmu,
                             scale=rstd)


def _body(nc, tc, t):
    with tc.tile_pool(name="const", bufs=1) as const:
        eps_t = const.tile([128, 1], F32)
        nc.vector.memset(eps_t, EPS)
        bq_t = const.tile([128, 8], F32)
        bk_t = const.tile([128, 8], F32)
        b1s_t = const.tile([128, 32], F32)

        with tc.tile_pool(name="ysbp", bufs=1) as ysbp, \
             tc.tile_pool(name="h2p", bufs=1) as h2p, \
             tc.tile_pool(name="ln2s", bufs=4) as stat2, \
             tc.tile_pool(name="ln2w", bufs=2) as wrk2:
            y_sb = ysbp.tile([128, 8, D], F32)
            h2sb = [h2p.tile([128, 8, 256], BF16, tag=f"h2sb{c}",
                             name=f"h2sb{c}") for c in range(4)]

            with tc.tile_pool(name="vaugp", bufs=1) as vaugp, \
                 tc.tile_pool(name="qfmp", bufs=1) as qfmp, \
                 tc.tile_pool(name="ksbp", bufs=1) as ksbp:
                vaug = vaugp.tile([128, 16, H, 65], BF16)
                q_fm = qfmp.tile([128, 8, OWN], BF16)
                ksb = ksbp.tile([128, 8, S], BF16)

                nc.vector.memset(vaug[:, :, :, 64:65], 1.0)

                # ===== P0+P1 fused: LN1 / V / K / Q =====
                with tc.tile_pool(name="p01h1", bufs=1) as h1p, \
                     tc.tile_pool(name="p01x", bufs=6) as xpool, \
                     tc.tile_pool(name="p01s", bufs=6) as stat, \
                     tc.tile_pool(name="p01wv", bufs=2) as wvp, \
                     tc.tile_pool(name="p01wst", bufs=5) as wstr, \
                     tc.tile_pool(name="p01vps", bufs=2, space="PSUM") as vps, \
                     tc.tile_pool(name="p01kps", bufs=2, space="PSUM") as kps, \
                     tc.tile_pool(name="p01qps", bufs=2, space="PSUM") as qps:
                    h1_fm = h1p.tile([128, 8, S], BF16)

                    def ln1_tile(tt, on_dve):
                        x_t = xpool.tile([128, D], BF16, tag="x", name="x_t")
                        nc.gpsimd.dma_start(
                            out=x_t, in_=t["xh"][tt * 128:(tt + 1) * 128, :])
                        _ln_normalize(nc, stat, eps_t, x_t, x_t, "a",
                                      on_dve=on_dve)
                        nc.sync.dma_start_transpose(
                            out=h1_fm[:, :, tt * 128:(tt + 1) * 128], in_=x_t)

                    def load_wv(fb):
                        wvh = wvp.tile([128, 8, 512], BF16, tag="wv", name="wvh")
                        nc.gpsimd.dma_start(out=wvh, in_=t["wvd"][:, fb, :, :])
                        return wvh

                    def v_tile(wvh, fb, tt, on_act):
                        ps = vps.tile([128, 512], F32, tag="ps", name="vps")
                        for kt in range(8):
                            nc.tensor.matmul(
                                ps, h1_fm[:, kt, tt * 128:(tt + 1) * 128],
                                wvh[:, kt, :],
                                start=(kt == 0), stop=(kt == 7))
                        dst = vaug[:, tt, fb * 8:(fb + 1) * 8, 0:64]
                        if on_act:
                            nc.scalar.copy(
                                out=dst, in_=ps.rearrange("p (h f) -> p h f", h=8))
                        else:
                            nc.vector.tensor_copy(
                                out=dst, in_=ps.rearrange("p (h f) -> p h f", h=8))

                    def k_pass(qbs, on_act):
                        for m in range(8):
                            wkm = wstr.tile([128, 8, 128], BF16, tag="w", name="wkm")
                            nc.sync.dma_start(
                                out=wkm,
                                in_=t["wk8"][m].rearrange("p (t n) -> p t n", t=8))
                            for qb in (qbs if isinstance(qbs, list) else [qbs]):
                                ps = kps.tile([128, 512], F32, tag="ps", name="kps")
                                for kt in range(8):
                                    nc.tensor.matmul(
                                        ps, wkm[:, kt, :],
                                        h1_fm[:, kt, qb * 512:(qb + 1) * 512],
                                        start=(kt == 0), stop=(kt == 7))
                                dst = ksb[:, m, qb * 512:(qb + 1) * 512]
                                if on_act:
                                    nc.scalar.activation(
                                        out=dst, in_=ps, func=AF.Identity,
                                        bias=bk_t[:, m:m + 1], scale=1.0)
                                else:
                                    nc.vector.tensor_scalar_add(
                                        out=dst, in0=ps,
                                        scalar1=bk_t[:, m:m + 1])

                    def q_pass(phases, on_act):
                        for m in range(8):
                            wqm = wstr.tile([128, 8, 128], BF16, tag="w", name="wqm")
                            nc.sync.dma_start(
                                out=wqm,
                                in_=t["wq8"][m].rearrange("p (t n) -> p t n", t=8))
                            for p in phases:
                                ps = qps.tile([128, 256], F32, tag="ps", name="qps")
                                for kt in range(8):
                                    nc.tensor.matmul(
                                        ps, wqm[:, kt, :],
                                        h1_fm[:, kt, 256 + 512 * p:512 + 512 * p],
                                        start=(kt == 0), stop=(kt == 7))
                                dst = q_fm[:, m, p * 256:(p + 1) * 256]
                                if on_act:
                                    nc.scalar.activation(
                                        out=dst, in_=ps, func=AF.Identity,
                                        bias=bq_t[:, m:m + 1], scale=1.0)
                                else:
                                    nc.vector.tensor_scalar_add(
                                        out=dst, in0=ps,
                                        scalar1=bq_t[:, m:m + 1])

                    for half in range(2):
                        t0 = 8 * half
                        on_act = (half == 0)
                        ln1_tile(t0, not on_act)
                        wvh0 = load_wv(0)
                        ln1_tile(t0 + 1, not on_act)
                        ln1_tile(t0 + 2, not on_act)
                        ln1_tile(t0 + 3, not on_act)
                        if half == 0:
                            nc.gpsimd.dma_start(out=bq_t, in_=t["bqd"])
                            nc.gpsimd.dma_start(out=bk_t, in_=t["bkd"])
                        for i in range(4):
                            ln1_tile(t0 + 4 + i, not on_act)
                            v_tile(wvh0, 0, t0 + i, on_act)
                        k_pass(2 * half, on_act)
                        for i in range(4, 8):
                            v_tile(wvh0, 0, t0 + i, on_act)
                        wvh1 = load_wv(1)
                        for i in range(8):
                            v_tile(wvh1, 1, t0 + i, on_act)
                        k_pass(2 * half + 1, on_act)
                        q_pass([2 * half, 2 * half + 1], on_act)
                    for p in range(4):
                        nc.sync.dma_start(
                            out=y_sb[:, 2 * p:2 * p + 2, :],
                            in_=bass.AP(tensor=t["xb"].tensor,
                                        offset=(256 + 512 * p) * D,
                                        ap=[[D, 128], [128 * D, 2], [1, D]]))

                # ===== P2: attention, 4 causal phases + fillers =====
                with tc.tile_pool(name="ctxp", bufs=1) as ctxp, \
                     tc.tile_pool(name="wop", bufs=1) as wop, \
                     tc.tile_pool(name="amsk", bufs=1) as mskp, \
                     tc.tile_pool(name="apt", bufs=3) as ptp, \
                     tc.tile_pool(name="anrm", bufs=4) as nrm, \
                     tc.tile_pool(name="aox", bufs=2) as oxp, \
                     tc.tile_pool(name="astps", bufs=2, space="PSUM") as stps, \
                     tc.tile_pool(name="acxps", bufs=1, space="PSUM") as cxps, \
                     tc.tile_pool(name="aops", bufs=1, space="PSUM") as opps:
                    ctx_fm = ctxp.tile([128, 8, OWN], BF16)
                    wo_t = wop.tile([128, 8, D], BF16)
                    bo_bc = wop.tile([128, D], F32, tag="bo", name="bo_bc")
                    mask_t = mskp.tile([128, 16, 256], BF16)
                    nc.gpsimd.dma_start(
                        out=mask_t,
                        in_=t["mskd"].rearrange("p (m q) -> p m q", m=16))
                    nc.gpsimd.dma_start(
                        out=wo_t, in_=t["wod"].rearrange("p (t n) -> p t n", t=8))
                    nc.gpsimd.dma_start(out=bo_bc, in_=_bcast(t["bo"]))

                    def ln2_tile(j):
                        h2_t = wrk2.tile([128, D], BF16, tag="h2t", name="h2t")
                        _ln_normalize(nc, stat2, eps_t, y_sb[:, j, :], h2_t, "b",
                                      on_dve=(j >= 2))
                        nc.sync.dma_start_transpose(
                            out=h2sb[j // 2][:, :, (j % 2) * 128:
                                             (j % 2) * 128 + 128], in_=h2_t)

                    def op_tile(j, n):
                        po = opps.tile([128, 512], F32, tag="po", name="po")
                        for kt in range(8):
                            nc.tensor.matmul(
                                po, ctx_fm[:, kt, j * 128:(j + 1) * 128],
                                wo_t[:, kt, n * 512:(n + 1) * 512],
                                start=(kt == 0), stop=(kt == 7))
                        yh = oxp.tile([128, 512], F32, tag="oy", name="yh")
                        nc.vector.tensor_add(
                            out=yh, in0=po, in1=bo_bc[:, n * 512:(n + 1) * 512])
                        nc.vector.tensor_add(
                            out=y_sb[:, j, n * 512:(n + 1) * 512],
                            in0=y_sb[:, j, n * 512:(n + 1) * 512], in1=yh)

                    pending = [None]  # deferred softmax finalize (pcx, p, h2)

                    def finalize_ctx():
                        if pending[0] is None:
                            return
                        pcx, fp, fh2 = pending[0]
                        pending[0] = None
                        for hh in range(2):
                            rec = nrm.tile([1, 256], F32, tag="rec", name="rec")
                            nc.vector.reciprocal(out=rec, in_=pcx[hh][64:65, :])
                            pb = nrm.tile([64, 256], F32, tag="pb", name="pb")
                            nc.gpsimd.partition_broadcast(pb, rec)
                            nc.vector.tensor_mul(
                                out=ctx_fm[hh * 64:(hh + 1) * 64, fh2,
                                           fp * 256:(fp + 1) * 256],
                                in0=pcx[hh][0:64, :], in1=pb)

                    def attn_phase(p, filler):
                        ext = EXT[p]
                        chunks = ext // 4
                        # masked chunk (diagonal, extra mask hop) first in the
                        # accumulation; short unmasked chain closes the group
                        order = [chunks - 1] + list(range(chunks - 1))
                        for h2 in range(8):
                            pcx = [cxps.tile([65, 256], F32, tag=f"cx{hh}",
                                             name=f"cx{hh}") for hh in range(2)]
                            pts = {}

                            def sc(hh, c):
                                pst = stps.tile([128, 4, 256], F32, tag="st",
                                                name="pst")
                                for j in range(4):
                                    kt = 4 * c + j
                                    nc.tensor.matmul(
                                        pst[:, j, :],
                                        ksb[hh * 64:(hh + 1) * 64, h2,
                                            kt * 128:(kt + 1) * 128],
                                        q_fm[hh * 64:(hh + 1) * 64, h2,
                                             p * 256:(p + 1) * 256])
                                pt = ptp.tile([128, 4, 256], BF16, tag="pt",
                                              name="pt")
                                nc.scalar.activation(out=pt, in_=pst, func=AF.Exp)
                                if c == chunks - 1:
                                    nc.vector.tensor_mul(
                                        out=pt, in0=pt,
                                        in1=mask_t[:, 4 * p:4 * p + 4, :])
                                pts[(hh, c)] = pt

                            def av(hh, ci):
                                h = 2 * h2 + hh
                                c = order[ci]
                                pt = pts.pop((hh, c))
                                for j in range(4):
                                    kt = 4 * c + j
                                    nc.tensor.matmul(
                                        pcx[hh], vaug[:, kt, h, :], pt[:, j, :],
                                        start=(ci == 0 and j == 0),
                                        stop=(ci == chunks - 1 and j == 3))

                            sc(0, order[0])
                            sc(1, order[0])
                            finalize_ctx()
                            filler(h2)
                            for ci in range(1, chunks):
                                av(0, ci - 1)
                                sc(0, order[ci])
                                av(1, ci - 1)
                                sc(1, order[ci])
                            av(0, chunks - 1)
                            av(1, chunks - 1)
                            pending[0] = (pcx, p, h2)

                    def mk_filler(g):
                        if g < 0:
                            return lambda h2: None
                        thunks = [
                            lambda: op_tile(2 * g, 0),
                            lambda: op_tile(2 * g, 1),
                            lambda: ln2_tile(2 * g),
                            lambda: op_tile(2 * g + 1, 0),
                            lambda: op_tile(2 * g + 1, 1),
                            lambda: ln2_tile(2 * g + 1),
                            None, None,
                        ]

                        def filler(h2):
                            th = thunks[h2]
                            if th is not None:
                                th()
                        return filler

                    for p in range(4):
                        attn_phase(p, mk_filler(p - 1))
                    finalize_ctx()
                    # group 3 out-proj + LN2 (ctx/wo die with this scope)
                    for j in (6, 7):
                        op_tile(j, 0)
                        op_tile(j, 1)
                        ln2_tile(j)

            # ===== P4: MLP (single-pass w1 stream, resident w2) =====
            with tc.tile_pool(name="mw2", bufs=1) as w2p, \
                 tc.tile_pool(name="mw1", bufs=4) as w1str, \
                 tc.tile_pool(name="macts", bufs=1) as acts, \
                 tc.tile_pool(name="mout", bufs=3) as out4, \
                 tc.tile_pool(name="mps1", bufs=3, space="PSUM") as ps41, \
                 tc.tile_pool(name="mps2", bufs=3, space="PSUM") as ps42:
                w2_t = w2p.tile([128, 32, D], BF16)
                b2_bc = w2p.tile([128, D], F32, tag="b2", name="b2_bc")
                nc.gpsimd.dma_start(out=b1s_t, in_=t["b1d"])
                nc.gpsimd.dma_start(out=b2_bc, in_=_bcast(t["b2"]))
                a_c = [acts.tile([128, 32, 256], BF16, tag=f"a{c}", name=f"a{c}")
                       for c in range(4)]

                def fc1_pass(cs):
                    for f in range(32):
                        w1f = w1str.tile([128, 8, 128], BF16, tag="w1",
                                         name="w1f")
                        nc.gpsimd.dma_start(
                            out=w1f,
                            in_=t["w1t"][f].rearrange("p (t n) -> p t n", t=8))
                        for c in cs:
                            ps = ps41.tile([128, 256], F32, tag="ps", name="ps")
                            for kt in range(8):
                                nc.tensor.matmul(
                                    ps, w1f[:, kt, :],
                                    h2sb[c][:, kt, :],
                                    start=(kt == 0), stop=(kt == 7))
                            nc.scalar.activation(out=a_c[c][:, f, :], in_=ps,
                                                 func=AF.Silu, scale=1.702,
                                                 bias=b1s_t[:, f:f + 1])
                        if f % 6 == 0 and f > 0 and f <= 24 and cs[0] == 0:
                            q = f // 6 - 1
                            nc.sync.dma_start(
                                out=w2_t[:, 8 * q:8 * (q + 1), :],
                                in_=t["w2d"][:, q * 8192:(q + 1) * 8192]
                                .rearrange("p (a c) -> p a c", a=8))

                def fc2_chunk(c):
                    for t2 in range(2):
                        j = 2 * c + t2
                        for n in range(2):
                            py = ps42.tile([128, 512], F32, tag="py", name="py")
                            for kt in range(32):
                                nc.tensor.matmul(
                                    py, a_c[c][:, kt, t2 * 128:(t2 + 1) * 128],
                                    w2_t[:, kt, n * 512:(n + 1) * 512],
                                    start=(kt == 0), stop=(kt == 31))
                            ot = out4.tile([128, 512], F32, tag="ot", name="ot")
                            nc.vector.tensor_add(
                                out=ot, in0=py,
                                in1=b2_bc[:, n * 512:(n + 1) * 512])
                            nc.vector.tensor_add(
                                out=ot, in0=ot,
                                in1=y_sb[:, j, n * 512:(n + 1) * 512])
                            nc.scalar.dma_start(
                                out=t["yo"][j * 128:(j + 1) * 128,
                                            n * 512:(n + 1) * 512],
                                in_=ot)

                fc1_pass([0, 1, 2])
                fc2_chunk(0)
                fc1_pass([3])
                fc2_chunk(1)
                fc2_chunk(2)
                fc2_chunk(3)


def _perms():
    g = [np.arange(256 * i, 256 * (i + 1)) for i in range(8)]
    order = [[1, 0, 2, 3, 5, 4, 6, 7], [0, 1, 3, 2, 4, 5, 7, 6]]
    return [np.concatenate([g[i] for i in o]) for o in order]


def _masks(perm):
    """[128, 16*256] bf16: slot 4p+i covers key tile EXT[p]-4+i of phase p."""
    m = np.zeros((16, 128, 256), np.float32)
    for p in range(4):
        qg = perm[256 + 512 * p:512 + 512 * p]
        for i in range(4):
            kt = EXT[p] - 4 + i
            kg = perm[kt * 128:(kt + 1) * 128]
            m[4 * p + i] = (kg[:, None] <= qg[None, :]).astype(np.float32)
    return np.ascontiguousarray(
        m.transpose(1, 0, 2).reshape(128, 16 * 256)).astype(ml_dtypes.bfloat16)


def _perm_w_mtiles(W, mt):
    """[Din, Dout] -> [mt, 128, Din//128 * (Dout//mt)]."""
    din, dout = W.shape
    n_sz = dout // mt
    A = W.reshape(din // 128, 128, mt, n_sz)
    return np.ascontiguousarray(A.transpose(2, 1, 0, 3).reshape(mt, 128, -1))


def _prep_consts(inputs):
    f = {k: np.asarray(v, np.float64) for k, v in inputs.items()}
    g1, b1 = f["ln1_g"], f["ln1_b"]
    g2, b2 = f["ln2_g"], f["ln2_b"]
    qs = 1.0 / np.sqrt(HD)
    wq = ((g1[:, None] * f["Wq"]) * qs).astype(np.float32)
    wk = (g1[:, None] * f["Wk"]).astype(np.float32)
    wv = (g1[:, None] * f["Wv"]).astype(np.float32)
    w1 = (g2[:, None] * f["W1"]).astype(np.float32)
    bf = ml_dtypes.bfloat16
    c = {}
    c["wq8"] = _perm_w_mtiles(wq, 8).astype(bf)
    c["wk8"] = _perm_w_mtiles(wk, 8).astype(bf)
    c["wvd"] = np.ascontiguousarray(
        wv.reshape(8, 128, 2, 512).transpose(1, 2, 0, 3)).astype(bf)
    c["wod"] = np.ascontiguousarray(
        f["Wo"].astype(np.float32).reshape(8, 128, D).transpose(1, 0, 2)
        .reshape(128, 8 * D)).astype(bf)
    c["w1t"] = _perm_w_mtiles(w1, 32).astype(bf)
    c["w2d"] = np.ascontiguousarray(
        (f["W2"] / 1.702).astype(np.float32)
        .reshape(32, 128, D).transpose(1, 0, 2).reshape(128, 32 * D)).astype(bf)
    c["bqd"] = np.ascontiguousarray(
        ((b1 @ f["Wq"] + f["bq"]) * qs).astype(np.float32).reshape(8, 128).T)
    c["bkd"] = np.ascontiguousarray(
        (b1 @ f["Wk"] + f["bk"]).astype(np.float32).reshape(8, 128).T)
    c["b1d"] = np.ascontiguousarray(
        (1.702 * (b2 @ f["W1"] + f["b1"])).astype(np.float32).reshape(32, 128).T)
    bv_eff = b1 @ f["Wv"] + f["bv"]
    c["bo"] = (bv_eff @ f["Wo"] + f["bo"]).astype(np.float32)
    c["b2"] = f["b2"].astype(np.float32)
    return c


def kernel(**inputs):
    if "nc" not in _CACHE:
        _CACHE["nc"] = _build_program()
        _CACHE["perms"] = _perms()
        _CACHE["masks"] = [_masks(p) for p in _CACHE["perms"]]
    nc = _CACHE["nc"]
    perms, masks = _CACHE["perms"], _CACHE["masks"]

    x = np.asarray(inputs["x"], np.float32)
    c = _prep_consts(inputs)

    in_maps = []
    for core in range(NCORES):
        b, s = core // 2, core % 2
        m = dict(c)
        m["xb"] = np.ascontiguousarray(x[b][perms[s]])
        m["xh"] = m["xb"].astype(ml_dtypes.bfloat16)
        m["mskd"] = masks[s]
        in_maps.append(m)

    res = run_bass_kernel_spmd(nc, in_maps, core_ids=list(range(NCORES)))

    own_local = np.concatenate(
        [np.arange(256 + 512 * p, 512 + 512 * p) for p in range(4)])
    out = np.empty((B, S, D), np.float32)
    for core in range(NCORES):
        b, s = core // 2, core % 2
        out[b][perms[s][own_local]] = res.results[core]["yo"]
    return out


# revision 7
# speedup vs baseline: 1.0552x; 1.0084x over previous
"""CLIP encoder layer (LN -> causal MHA -> residual -> LN -> quickGELU MLP -> residual)
on 8 Trainium2 NeuronCores, SPMD via bass/Tile. v2.

Sharding: 8 shards = 4 batches x 2 query-groups. Core c handles batch c//2 with
parity s = c%2, owning 1024 query tokens in 4 phase-groups of 256. Each core
recomputes LN1 + K/V for the full sequence of its batch (no collectives).

4-phase causal schedule: phase p attends its 256 own queries against the first
EXT[p] = (4,8,12,16) key tiles of 128. A per-parity permutation of 256-token
blocks (pairs swapped) makes the local key order prefix-consistent for both
parities while own queries sit at uniform local offsets 256+512p. Only the last
4 key tiles of each phase are masked (diagonal + parity waste), via data.

Everything on the matmul path is bf16 (tolerance 2e-2 allows it): LN outputs
bf16, transposed token-major -> feature-major by the DMA XBAR transpose (14ns/
tile, zero engine cost) instead of PE transposes. K stays resident in SBUF (no
DRAM spill), y (residual stream) stays resident in SBUF. Softmax uses the
augmented-V ones-row trick for sums; the reciprocal is broadcast across
partitions by GPSIMD partition_broadcast (no PE/DVE broadcast work). Weights
are pre-permuted on the host; LN affines, q-scale and quickGELU's 1.702 are
folded into weights/biases.
"""

import sys

sys.path.insert(0, "/opt/trn_rl_repo")

import numpy as np
import ml_dtypes

import concourse.bass as bass
import concourse.mybir as mybir
import concourse.tile as tile
from concourse import bacc
from concourse.bass_utils import run_bass_kernel_spmd

B, S, D = 4, 2048, 1024
H, HD = 16, 64
NCORES = 8
EPS = 1e-5
OWN = 1024
EXT = [4, 8, 12, 16]
F32 = mybir.dt.float32
BF16 = mybir.dt.bfloat16
FP8 = mybir.dt.float8e4
ALU = mybir.AluOpType
AF = mybir.ActivationFunctionType

_CACHE = {}


def _bcast(ap1d, p=128):
    """[N] DRAM vector viewed as [p, N] with partition-step 0 (broadcast load)."""
    return bass.AP(tensor=ap1d.tensor, offset=ap1d.offset, ap=[[0, p]] + list(ap1d.ap))


def _build_program():
    nc = bacc.Bacc("TRN2", target_bir_lowering=False, debug=False,
                   num_devices=NCORES)

    t = {}
    t["xb"] = nc.dram_tensor("xb", [S, D], F32, kind="ExternalInput").ap()
    t["xh"] = nc.dram_tensor("xh", [S, D], BF16, kind="ExternalInput").ap()
    t["wq8"] = nc.dram_tensor("wq8", [8, 128, D], BF16, kind="ExternalInput").ap()
    t["wk8"] = nc.dram_tensor("wk8", [8, 128, D], BF16, kind="ExternalInput").ap()
    t["wvd"] = nc.dram_tensor("wvd", [128, 2, 8, 512], BF16, kind="ExternalInput").ap()
    t["wod"] = nc.dram_tensor("wod", [128, 8 * D], BF16, kind="ExternalInput").ap()
    t["w1t"] = nc.dram_tensor("w1t", [32, 128, D], BF16, kind="ExternalInput").ap()
    t["w2d"] = nc.dram_tensor("w2d", [128, 32 * D], BF16, kind="ExternalInput").ap()
    t["mskd"] = nc.dram_tensor("mskd", [128, 16 * 256], BF16, kind="ExternalInput").ap()
    t["bqd"] = nc.dram_tensor("bqd", [128, 8], F32, kind="ExternalInput").ap()
    t["bkd"] = nc.dram_tensor("bkd", [128, 8], F32, kind="ExternalInput").ap()
    t["b1d"] = nc.dram_tensor("b1d", [128, 32], F32, kind="ExternalInput").ap()
    for b_ in ("bo", "b2"):
        t[b_] = nc.dram_tensor(b_, [D], F32, kind="ExternalInput").ap()
    t["yo"] = nc.dram_tensor("yo", [OWN, D], F32, kind="ExternalOutput").ap()

    with tile.TileContext(nc) as tc:
        _body(nc, tc, t)
    nc.compile()
    return nc


def _ln_normalize(nc, stat, eps_t, x_t, out_t, tag, on_dve=False):
    """LayerNorm (affine folded into downstream weights): out = (x-mu)*rstd.
    Stats on DVE; the big normalize pass on ACT, or on DVE (tensor_scalar)
    when ACT is the busier engine (attention-phase LN2)."""
    st = stat.tile([128, 2, 6], F32, tag=f"{tag}st")
    for g in range(2):
        nc.vector.bn_stats(out=st[:, g, :], in_=x_t[:, g * 512:(g + 1) * 512])
    mv = stat.tile([128, 2], F32, tag=f"{tag}mv")
    nc.vector.bn_aggr(out=mv, in_=st)
    rstd = stat.tile([128, 1], F32, tag=f"{tag}rs")
    nc.scalar.activation(out=rstd, in_=mv[:, 1:2], func=AF.Sqrt, bias=eps_t, scale=1.0)
    nc.vector.reciprocal(out=rstd, in_=rstd)
    nmu = stat.tile([128, 1], F32, tag=f"{tag}nm")
    nc.vector.tensor_scalar(out=nmu, in0=mv[:, 0:1], scalar1=rstd, scalar2=-1.0,
                            op0=ALU.mult, op1=ALU.mult)
    if on_dve:
        nc.vector.tensor_scalar(out=out_t, in0=x_t, scalar1=rstd, scalar2=nmu,
                                op0=ALU.mult, op1=ALU.add)
    else:
        nc.scalar.activation(out=out_t, in_=x_t, func=AF.Identity, bias=nmu,
                             scale=rstd)


def _body(nc, tc, t):
    with tc.tile_pool(name="const", bufs=1) as const:
        eps_t = const.tile([128, 1], F32)
        nc.vector.memset(eps_t, EPS)
        bq_t = const.tile([128, 8], F32)
        bk_t = const.tile([128, 8], F32)
        b1s_t = const.tile([128, 32], F32)

        with tc.tile_pool(name="ysbp", bufs=1) as ysbp, \
             tc.tile_pool(name="h2p", bufs=1) as h2p, \
             tc.tile_pool(name="ln2s", bufs=6) as stat2, \
             tc.tile_pool(name="ln2w", bufs=3) as wrk2:
            y_sb = ysbp.tile([128, 8, D], F32)
            h2sb = [h2p.tile([128, 8, 256], BF16, tag=f"h2sb{c}",
                             name=f"h2sb{c}") for c in range(4)]

            with tc.tile_pool(name="vaugp", bufs=1) as vaugp, \
                 tc.tile_pool(name="qfmp", bufs=1) as qfmp, \
                 tc.tile_pool(name="ksbp", bufs=1) as ksbp:
                vaug = vaugp.tile([128, 16, H, 65], BF16)
                q_fm = qfmp.tile([128, 8, OWN], BF16)
                ksb = ksbp.tile([128, 8, S], BF16)

                nc.vector.memset(vaug[:, :, :, 64:65], 1.0)

                # ===== P0+P1 fused: LN1 / V / K / Q =====
                with tc.tile_pool(name="p01h1", bufs=1) as h1p, \
                     tc.tile_pool(name="p01x", bufs=6) as xpool, \
                     tc.tile_pool(name="p01s", bufs=6) as stat, \
                     tc.tile_pool(name="p01wv", bufs=2) as wvp, \
                     tc.tile_pool(name="p01wst", bufs=5) as wstr, \
                     tc.tile_pool(name="p01vps", bufs=2, space="PSUM") as vps, \
                     tc.tile_pool(name="p01kps", bufs=2, space="PSUM") as kps, \
                     tc.tile_pool(name="p01qps", bufs=2, space="PSUM") as qps:
                    h1_fm = h1p.tile([128, 8, S], BF16)

                    def ln1_tile(tt, on_dve):
                        x_t = xpool.tile([128, D], BF16, tag="x", name="x_t")
                        xq = nc.sync if tt < 2 else nc.gpsimd
                        xq.dma_start(
                            out=x_t, in_=t["xh"][tt * 128:(tt + 1) * 128, :])
                        _ln_normalize(nc, stat, eps_t, x_t, x_t, "a",
                                      on_dve=on_dve)
                        nc.sync.dma_start_transpose(
                            out=h1_fm[:, :, tt * 128:(tt + 1) * 128], in_=x_t)

                    def load_wv(fb):
                        wvh = wvp.tile([128, 8, 512], BF16, tag="wv", name="wvh")
                        nc.gpsimd.dma_start(out=wvh, in_=t["wvd"][:, fb, :, :])
                        return wvh

                    def v_tile(wvh, fb, tt, on_act):
                        ps = vps.tile([128, 512], F32, tag="ps", name="vps")
                        for kt in range(8):
                            nc.tensor.matmul(
                                ps, h1_fm[:, kt, tt * 128:(tt + 1) * 128],
                                wvh[:, kt, :],
                                start=(kt == 0), stop=(kt == 7))
                        dst = vaug[:, tt, fb * 8:(fb + 1) * 8, 0:64]
                        if on_act:
                            nc.scalar.copy(
                                out=dst, in_=ps.rearrange("p (h f) -> p h f", h=8))
                        else:
                            nc.vector.tensor_copy(
                                out=dst, in_=ps.rearrange("p (h f) -> p h f", h=8))

                    def k_pass(qbs, on_act):
                        for m in range(8):
                            wkm = wstr.tile([128, 8, 128], BF16, tag="w", name="wkm")
                            nc.sync.dma_start(
                                out=wkm,
                                in_=t["wk8"][m].rearrange("p (t n) -> p t n", t=8))
                            for qb in (qbs if isinstance(qbs, list) else [qbs]):
                                ps = kps.tile([128, 512], F32, tag="ps", name="kps")
                                for kt in range(8):
                                    nc.tensor.matmul(
                                        ps, wkm[:, kt, :],
                                        h1_fm[:, kt, qb * 512:(qb + 1) * 512],
                                        start=(kt == 0), stop=(kt == 7))
                                dst = ksb[:, m, qb * 512:(qb + 1) * 512]
                                if on_act:
                                    nc.scalar.activation(
                                        out=dst, in_=ps, func=AF.Identity,
                                        bias=bk_t[:, m:m + 1], scale=1.0)
                                else:
                                    nc.vector.tensor_scalar_add(
                                        out=dst, in0=ps,
                                        scalar1=bk_t[:, m:m + 1])

                    def q_pass(phases, on_act):
                        for m in range(8):
                            wqm = wstr.tile([128, 8, 128], BF16, tag="w", name="wqm")
                            nc.sync.dma_start(
                                out=wqm,
                                in_=t["wq8"][m].rearrange("p (t n) -> p t n", t=8))
                            for p in phases:
                                ps = qps.tile([128, 256], F32, tag="ps", name="qps")
                                for kt in range(8):
                                    nc.tensor.matmul(
                                        ps, wqm[:, kt, :],
                                        h1_fm[:, kt, 256 + 512 * p:512 + 512 * p],
                                        start=(kt == 0), stop=(kt == 7))
                                dst = q_fm[:, m, p * 256:(p + 1) * 256]
                                if on_act:
                                    nc.scalar.activation(
                                        out=dst, in_=ps, func=AF.Identity,
                                        bias=bq_t[:, m:m + 1], scale=1.0)
                                else:
                                    nc.vector.tensor_scalar_add(
                                        out=dst, in0=ps,
                                        scalar1=bq_t[:, m:m + 1])

                    for half in range(2):
                        t0 = 8 * half
                        on_act = (half == 0)
                        ln1_tile(t0, not on_act)
                        wvh0 = load_wv(0)
                        ln1_tile(t0 + 1, not on_act)
                        ln1_tile(t0 + 2, not on_act)
                        ln1_tile(t0 + 3, not on_act)
                        if half == 0:
                            nc.gpsimd.dma_start(out=bq_t, in_=t["bqd"])
                            nc.gpsimd.dma_start(out=bk_t, in_=t["bkd"])
                        for i in range(4):
                            ln1_tile(t0 + 4 + i, not on_act)
                            v_tile(wvh0, 0, t0 + i, on_act)
                        k_pass(2 * half, on_act)
                        for i in range(4, 8):
                            v_tile(wvh0, 0, t0 + i, on_act)
                        wvh1 = load_wv(1)
                        for i in range(8):
                            v_tile(wvh1, 1, t0 + i, on_act)
                        k_pass(2 * half + 1, on_act)
                        q_pass([2 * half, 2 * half + 1], on_act)
                    for p in range(4):
                        nc.sync.dma_start(
                            out=y_sb[:, 2 * p:2 * p + 2, :],
                            in_=bass.AP(tensor=t["xb"].tensor,
                                        offset=(256 + 512 * p) * D,
                                        ap=[[D, 128], [128 * D, 2], [1, D]]))

                # ===== P2: attention, 4 causal phases + fillers =====
                with tc.tile_pool(name="ctxp", bufs=1) as ctxp, \
                     tc.tile_pool(name="wop", bufs=1) as wop, \
                     tc.tile_pool(name="amsk", bufs=1) as mskp, \
                     tc.tile_pool(name="apt", bufs=4) as ptp, \
                     tc.tile_pool(name="anrm", bufs=6) as nrm, \
                     tc.tile_pool(name="aox", bufs=3) as oxp, \
                     tc.tile_pool(name="astps", bufs=2, space="PSUM") as stps, \
                     tc.tile_pool(name="acxps", bufs=1, space="PSUM") as cxps, \
                     tc.tile_pool(name="aops", bufs=1, space="PSUM") as opps:
                    ctx_fm = ctxp.tile([128, 8, OWN], BF16)
                    wo_t = wop.tile([128, 8, D], BF16)
                    bo_bc = wop.tile([128, D], F32, tag="bo", name="bo_bc")
                    mask_t = mskp.tile([128, 16, 256], BF16)
                    nc.gpsimd.dma_start(
                        out=mask_t,
                        in_=t["mskd"].rearrange("p (m q) -> p m q", m=16))
                    nc.gpsimd.dma_start(
                        out=wo_t, in_=t["wod"].rearrange("p (t n) -> p t n", t=8))
                    nc.gpsimd.dma_start(out=bo_bc, in_=_bcast(t["bo"]))

                    def ln2_tile(j):
                        h2_t = wrk2.tile([128, D], BF16, tag="h2t", name="h2t")
                        _ln_normalize(nc, stat2, eps_t, y_sb[:, j, :], h2_t, "b",
                                      on_dve=(j >= 2))
                        nc.sync.dma_start_transpose(
                            out=h2sb[j // 2][:, :, (j % 2) * 128:
                                             (j % 2) * 128 + 128], in_=h2_t)

                    def op_tile(j, n):
                        po = opps.tile([128, 512], F32, tag="po", name="po")
                        for kt in range(8):
                            nc.tensor.matmul(
                                po, ctx_fm[:, kt, j * 128:(j + 1) * 128],
                                wo_t[:, kt, n * 512:(n + 1) * 512],
                                start=(kt == 0), stop=(kt == 7))
                        yh = oxp.tile([128, 512], F32, tag="oy", name="yh")
                        nc.vector.tensor_add(
                            out=yh, in0=po, in1=bo_bc[:, n * 512:(n + 1) * 512])
                        nc.vector.tensor_add(
                            out=y_sb[:, j, n * 512:(n + 1) * 512],
                            in0=y_sb[:, j, n * 512:(n + 1) * 512], in1=yh)

                    pending = [None]  # deferred softmax finalize (pcx, p, h2)

                    def finalize_ctx():
                        if pending[0] is None:
                            return
                        pcx, fp, fh2 = pending[0]
                        pending[0] = None
                        for hh in range(2):
                            rec = nrm.tile([1, 256], F32, tag="rec", name="rec")
                            nc.vector.reciprocal(out=rec, in_=pcx[hh][64:65, :])
                            pb = nrm.tile([64, 256], F32, tag="pb", name="pb")
                            nc.gpsimd.partition_broadcast(pb, rec)
                            nc.vector.tensor_mul(
                                out=ctx_fm[hh * 64:(hh + 1) * 64, fh2,
                                           fp * 256:(fp + 1) * 256],
                                in0=pcx[hh][0:64, :], in1=pb)

                    def attn_phase(p, filler):
                        ext = EXT[p]
                        chunks = ext // 4
                        # masked chunk (diagonal, extra mask hop) first in the
                        # accumulation; short unmasked chain closes the group
                        order = [chunks - 1] + list(range(chunks - 1))
                        for h2 in range(8):
                            pcx = [cxps.tile([65, 256], F32, tag=f"cx{hh}",
                                             name=f"cx{hh}") for hh in range(2)]
                            pts = {}

                            def sc(hh, c):
                                pst = stps.tile([128, 4, 256], F32, tag="st",
                                                name="pst")
                                for j in range(4):
                                    kt = 4 * c + j
                                    nc.tensor.matmul(
                                        pst[:, j, :],
                                        ksb[hh * 64:(hh + 1) * 64, h2,
                                            kt * 128:(kt + 1) * 128],
                                        q_fm[hh * 64:(hh + 1) * 64, h2,
                                             p * 256:(p + 1) * 256])
                                pt = ptp.tile([128, 4, 256], BF16, tag="pt",
                                              name="pt")
                                nc.scalar.activation(out=pt, in_=pst, func=AF.Exp)
                                if c == chunks - 1:
                                    nc.vector.tensor_mul(
                                        out=pt, in0=pt,
                                        in1=mask_t[:, 4 * p:4 * p + 4, :])
                                pts[(hh, c)] = pt

                            def av(hh, ci):
                                h = 2 * h2 + hh
                                c = order[ci]
                                pt = pts.pop((hh, c))
                                for j in range(4):
                                    kt = 4 * c + j
                                    nc.tensor.matmul(
                                        pcx[hh], vaug[:, kt, h, :], pt[:, j, :],
                                        start=(ci == 0 and j == 0),
                                        stop=(ci == chunks - 1 and j == 3))

                            sc(0, order[0])
                            sc(1, order[0])
                            finalize_ctx()
                            filler(h2)
                            for ci in range(1, chunks):
                                av(0, ci - 1)
                                sc(0, order[ci])
                                av(1, ci - 1)
                                sc(1, order[ci])
                            av(0, chunks - 1)
                            av(1, chunks - 1)
                            pending[0] = (pcx, p, h2)

                    def mk_filler(g):
                        if g < 0:
                            return lambda h2: None
                        thunks = [
                            lambda: op_tile(2 * g, 0),
                            lambda: op_tile(2 * g, 1),
                            lambda: ln2_tile(2 * g),
                            lambda: op_tile(2 * g + 1, 0),
                            lambda: op_tile(2 * g + 1, 1),
                            lambda: ln2_tile(2 * g + 1),
                            None, None,
                        ]

                        def filler(h2):
                            th = thunks[h2]
                            if th is not None:
                                th()
                        return filler

                    for p in range(4):
                        attn_phase(p, mk_filler(p - 1))
                    finalize_ctx()
                    # group 3 out-proj + LN2 (ctx/wo die with this scope)
                    for j in (6, 7):
                        op_tile(j, 0)
                        op_tile(j, 1)
                        ln2_tile(j)

            # ===== P4: MLP (single-pass w1 stream, resident w2) =====
            with tc.tile_pool(name="mw2", bufs=1) as w2p, \
                 tc.tile_pool(name="mw1", bufs=5) as w1str, \
                 tc.tile_pool(name="macts", bufs=1) as acts, \
                 tc.tile_pool(name="mout", bufs=3) as out4, \
                 tc.tile_pool(name="mps1", bufs=3, space="PSUM") as ps41, \
                 tc.tile_pool(name="mps2", bufs=3, space="PSUM") as ps42:
                w2_t = w2p.tile([128, 32, D], BF16)
                b2_bc = w2p.tile([128, D], F32, tag="b2", name="b2_bc")
                nc.gpsimd.dma_start(out=b1s_t, in_=t["b1d"])
                nc.gpsimd.dma_start(out=b2_bc, in_=_bcast(t["b2"]))
                a_c = [acts.tile([128, 32, 256], BF16, tag=f"a{c}", name=f"a{c}")
                       for c in range(4)]

                def fc1_pass(cs):
                    for f in range(32):
                        w1f = w1str.tile([128, 8, 128], BF16, tag="w1",
                                         name="w1f")
                        nc.gpsimd.dma_start(
                            out=w1f,
                            in_=t["w1t"][f].rearrange("p (t n) -> p t n", t=8))
                        for c in cs:
                            ps = ps41.tile([128, 256], F32, tag="ps", name="ps")
                            for kt in range(8):
                                nc.tensor.matmul(
                                    ps, w1f[:, kt, :],
                                    h2sb[c][:, kt, :],
                                    start=(kt == 0), stop=(kt == 7))
                            nc.scalar.activation(out=a_c[c][:, f, :], in_=ps,
                                                 func=AF.Silu, scale=1.702,
                                                 bias=b1s_t[:, f:f + 1])
                        if f % 6 == 0 and f > 0 and f <= 24 and cs[0] == 0:
                            q = f // 6 - 1
                            nc.sync.dma_start(
                                out=w2_t[:, 8 * q:8 * (q + 1), :],
                                in_=t["w2d"][:, q * 8192:(q + 1) * 8192]
                                .rearrange("p (a c) -> p a c", a=8))

                def fc2_chunk(c):
                    for t2 in range(2):
                        j = 2 * c + t2
                        for n in range(2):
                            py = ps42.tile([128, 512], F32, tag="py", name="py")
                            for kt in range(32):
                                nc.tensor.matmul(
                                    py, a_c[c][:, kt, t2 * 128:(t2 + 1) * 128],
                                    w2_t[:, kt, n * 512:(n + 1) * 512],
                                    start=(kt == 0), stop=(kt == 31))
                            ot = out4.tile([128, 512], F32, tag="ot", name="ot")
                            nc.vector.tensor_add(
                                out=ot, in0=py,
                                in1=b2_bc[:, n * 512:(n + 1) * 512])
                            nc.vector.tensor_add(
                                out=ot, in0=ot,
                                in1=y_sb[:, j, n * 512:(n + 1) * 512])
                            nc.scalar.dma_start(
                                out=t["yo"][j * 128:(j + 1) * 128,
                                            n * 512:(n + 1) * 512],
                                in_=ot)

                fc1_pass([0, 1, 2])
                fc2_chunk(0)
                fc1_pass([3])
                fc2_chunk(1)
                fc2_chunk(2)
                fc2_chunk(3)


def _perms():
    g = [np.arange(256 * i, 256 * (i + 1)) for i in range(8)]
    order = [[1, 0, 2, 3, 5, 4, 6, 7], [0, 1, 3, 2, 4, 5, 7, 6]]
    return [np.concatenate([g[i] for i in o]) for o in order]


def _masks(perm):
    """[128, 16*256] bf16: slot 4p+i covers key tile EXT[p]-4+i of phase p."""
    m = np.zeros((16, 128, 256), np.float32)
    for p in range(4):
        qg = perm[256 + 512 * p:512 + 512 * p]
        for i in range(4):
            kt = EXT[p] - 4 + i
            kg = perm[kt * 128:(kt + 1) * 128]
            m[4 * p + i] = (kg[:, None] <= qg[None, :]).astype(np.float32)
    return np.ascontiguousarray(
        m.transpose(1, 0, 2).reshape(128, 16 * 256)).astype(ml_dtypes.bfloat16)


def _perm_w_mtiles(W, mt):
    """[Din, Dout] -> [mt, 128, Din//128 * (Dout//mt)]."""
    din, dout = W.shape
    n_sz = dout // mt
    A = W.reshape(din // 128, 128, mt, n_sz)
    return np.ascontiguousarray(A.transpose(2, 1, 0, 3).reshape(mt, 128, -1))


def _prep_consts(inputs):
    f = {k: np.asarray(v, np.float64) for k, v in inputs.items()}
    g1, b1 = f["ln1_g"], f["ln1_b"]
    g2, b2 = f["ln2_g"], f["ln2_b"]
    qs = 1.0 / np.sqrt(HD)
    wq = ((g1[:, None] * f["Wq"]) * qs).astype(np.float32)
    wk = (g1[:, None] * f["Wk"]).astype(np.float32)
    wv = (g1[:, None] * f["Wv"]).astype(np.float32)
    w1 = (g2[:, None] * f["W1"]).astype(np.float32)
    bf = ml_dtypes.bfloat16
    c = {}
    c["wq8"] = _perm_w_mtiles(wq, 8).astype(bf)
    c["wk8"] = _perm_w_mtiles(wk, 8).astype(bf)
    c["wvd"] = np.ascontiguousarray(
        wv.reshape(8, 128, 2, 512).transpose(1, 2, 0, 3)).astype(bf)
    c["wod"] = np.ascontiguousarray(
        f["Wo"].astype(np.float32).reshape(8, 128, D).transpose(1, 0, 2)
        .reshape(128, 8 * D)).astype(bf)
    c["w1t"] = _perm_w_mtiles(w1, 32).astype(bf)
    c["w2d"] = np.ascontiguousarray(
        (f["W2"] / 1.702).astype(np.float32)
        .reshape(32, 128, D).transpose(1, 0, 2).reshape(128, 32 * D)).astype(bf)
    c["bqd"] = np.ascontiguousarray(
        ((b1 @ f["Wq"] + f["bq"]) * qs).astype(np.float32).reshape(8, 128).T)
    c["bkd"] = np.ascontiguousarray(
        (b1 @ f["Wk"] + f["bk"]).astype(np.float32).reshape(8, 128).T)
    c["b1d"] = np.ascontiguousarray(
        (1.702 * (b2 @ f["W1"] + f["b1"])).astype(np.float32).reshape(32, 128).T)
    bv_eff = b1 @ f["Wv"] + f["bv"]
    c["bo"] = (bv_eff @ f["Wo"] + f["bo"]).astype(np.float32)
    c["b2"] = f["b2"].astype(np.float32)
    return c


def kernel(**inputs):
    if "nc" not in _CACHE:
        _CACHE["nc"] = _build_program()
        _CACHE["perms"] = _perms()
        _CACHE["masks"] = [_masks(p) for p in _CACHE["perms"]]
    nc = _CACHE["nc"]
    perms, masks = _CACHE["perms"], _CACHE["masks"]

    x = np.asarray(inputs["x"], np.float32)
    c = _prep_consts(inputs)

    in_maps = []
    for core in range(NCORES):
        b, s = core // 2, core % 2
        m = dict(c)
        m["xb"] = np.ascontiguousarray(x[b][perms[s]])
        m["xh"] = m["xb"].astype(ml_dtypes.bfloat16)
        m["mskd"] = masks[s]
        in_maps.append(m)

    res = run_bass_kernel_spmd(nc, in_maps, core_ids=list(range(NCORES)))

    own_local = np.concatenate(
        [np.arange(256 + 512 * p, 512 + 512 * p) for p in range(4)])
    out = np.empty((B, S, D), np.float32)
    for core in range(NCORES):
        b, s = core // 2, core % 2
        out[b][perms[s][own_local]] = res.results[core]["yo"]
    return out
